# revision 12
# baseline (speedup 1.0000x reference)
"""Self-contained Bass/Tile SPMD kernel for nn_AIA_1_56049323213170 (8 NeuronCores).

Pipeline (B=2, C=256, H=W=128), all heavy math on-device in bf16/f32-psum:
  M1 = Xc @ Xr (CxC, contraction sharded 8-way + AllReduce)
  a  = rowsoftmax(M1)  (redundant per core, unnormalized + row-recip trick)
  s  = a @ Xc          (sharded: rows by batch, cols by quarter -> (128, 8192))
  rowsoftmax(s) needs only a global row-sum (|s|<=5.5 -> shift-free exp):
       AllReduce of per-core row sums within each batch group of 4 cores
  cia = x + softmax_W(rowsoftmax(s))   (W-softmax local, shift-free)
  AllGather cia within batch group (split in 2 halves for pipelining)
  y   = BN(conv3x3s2(x)+b), x1_2 = relu(BN(conv3x3s2(cia)+b))  (64 out-ch/core)
  branch = relu(y); x4_3 = sigmoid(lrelu(y,.2))
  att1 = rowsoftmax(x1_2 @ branch^T); att2 = rowsoftmax(branch @ x4_3^T)
  x3_3 = rowsoftmax(x1_2 @ att2^T)  (att2/x3_3 shift-free)
  out  = bilinear_up2(relu(x3_3 + att1 + branch)) via R @ S @ R^T

v2: overlap-oriented schedule (conv-x + att2 prep run during the collective
chain), unpadded conv inputs with ragged edge taps (contiguous DMA), and a
PSUM-quadrant-packed attention tail (two channel-pairs per instruction).

Core p: b = p//4 (batch), q = p%4 (quarter; parity=q//2, h-half=q%2).
Each core returns out[b, 64q:64q+64] as bf16; host assembles f32.
"""
import numpy as np
import ml_dtypes

N_CORES = 8
B, C, H, W = 2, 256, 128, 128
HO = WO = 64
EPS = 1e-5


def _resize_mat(n_out, n_in):
    R = np.zeros((n_out, n_in), np.float32)
    scale = n_in / n_out
    for i in range(n_out):
        src = (i + 0.5) * scale - 0.5
        i0 = int(np.floor(src))
        frac = src - i0
        lo = min(max(i0, 0), n_in - 1)
        hi = min(max(i0 + 1, 0), n_in - 1)
        R[i, lo] += 1.0 - frac
        R[i, hi] += frac
    return R


def _ap_of(t):
    import concourse.bass as bass
    if isinstance(t, bass.AP):
        return t
    return t.ap()


def _bcast_last(t, n):
    """AP of tile t broadcast with a 0-step innermost dim of size n."""
    import concourse.bass as bass
    base = _ap_of(t)
    return bass.AP(tensor=base.tensor, offset=base.offset,
                   ap=[list(d) for d in base.ap] + [[0, n]])


def build_nc():
    from contextlib import ExitStack
    import concourse.bass as bass
    import concourse.mybir as mybir
    import concourse.tile as tile
    from concourse import bacc
    from concourse.masks import make_identity

    f32 = mybir.dt.float32
    bf16 = mybir.dt.bfloat16
    AF = mybir.ActivationFunctionType
    AX = mybir.AxisListType
    ALU = mybir.AluOpType

    nc = bacc.Bacc("TRN2", target_bir_lowering=False, debug=False,
                   num_devices=N_CORES)

    # ---- I/O ----
    xb = nc.dram_tensor("xb", [C, H, W], bf16, kind="ExternalInput")
    xcT = nc.dram_tensor("xcT", [4096, 256], bf16, kind="ExternalInput")
    xr = nc.dram_tensor("xr", [4096, 256], bf16, kind="ExternalInput")
    xcm2 = nc.dram_tensor("xcm2", [128, 2, 8192], bf16, kind="ExternalInput")
    xblk = nc.dram_tensor("xblk", [128, 8192], bf16, kind="ExternalInput")
    sel = nc.dram_tensor("sel", [128, 2, 128], bf16, kind="ExternalInput")
    wt = nc.dram_tensor("wt", [128, 2, 9, 64], bf16, kind="ExternalInput")
    scale_d = nc.dram_tensor("scale", [64, 1], f32, kind="ExternalInput")
    shift_d = nc.dram_tensor("shift", [64, 1], f32, kind="ExternalInput")
    rt_d = nc.dram_tensor("rt", [128, 128], bf16, kind="ExternalInput")
    wneg = nc.dram_tensor("wneg", [128, 2, 3, 64], bf16, kind="ExternalInput")
    out_sh = nc.dram_tensor("out_sh", [64, H, W], bf16, kind="ExternalOutput")

    # ---- collective scratch ----
    cc1_in = nc.dram_tensor("cc1_in", [256, 256], f32)
    cc1_out = nc.dram_tensor("cc1_out", [256, 256], f32, addr_space="Shared")
    cc2_in = nc.dram_tensor("cc2_in", [128, 1], f32)
    cc2_out = nc.dram_tensor("cc2_out", [128, 1], f32)
    cca_in = nc.dram_tensor("cca_in", [128, 4096], bf16)
    cca_out = nc.dram_tensor("cca_out", [4, 128, 4096], bf16)
    ccb_in = nc.dram_tensor("ccb_in", [128, 4096], bf16)
    ccb_out = nc.dram_tensor("ccb_out", [4, 128, 4096], bf16)
    G8 = [list(range(8))]
    G4 = [[0, 1, 2, 3], [4, 5, 6, 7]]

    with tile.TileContext(nc) as tc, ExitStack() as ctx:
        consts = ctx.enter_context(tc.tile_pool(name="consts", bufs=1))
        awork = ctx.enter_context(tc.tile_pool(name="awork", bufs=1))
        big = ctx.enter_context(tc.tile_pool(name="big", bufs=3))
        xpool = ctx.enter_context(tc.tile_pool(name="xpool", bufs=3))
        convin = ctx.enter_context(tc.tile_pool(name="convin", bufs=2))
        convout = ctx.enter_context(tc.tile_pool(name="convout", bufs=1))
        attn = ctx.enter_context(tc.tile_pool(name="attn", bufs=2))
        psB = ctx.enter_context(tc.tile_pool(name="psB", bufs=4, space="PSUM"))
        psA_cm = tc.tile_pool(name="psA", bufs=3, space="PSUM")
        psA = psA_cm.__enter__()

        def psa():
            return psA.tile([128, 512], f32, tag="psA", name="psA_t")

        def psbf():
            return psB.tile([128, 512], f32, tag="psB", name="psB_t")

        # ================= constants =================
        id128 = consts.tile([128, 128], bf16, tag="id128")
        make_identity(nc, id128)
        rt_sb = consts.tile([128, 128], bf16, tag="rt")
        nc.sync.dma_start(out=rt_sb, in_=rt_d.ap())
        sel_sb = consts.tile([128, 2, 128], bf16, tag="sel")
        nc.sync.dma_start(out=sel_sb, in_=sel.ap())
        wt_sb = consts.tile([128, 2, 9, 64], bf16, tag="wt")
        nc.sync.dma_start(out=wt_sb, in_=wt.ap())
        scale_sb = consts.tile([64, 1], f32, tag="scale")
        nc.sync.dma_start(out=scale_sb, in_=scale_d.ap())
        shift_sb = consts.tile([64, 1], f32, tag="shift")
        nc.sync.dma_start(out=shift_sb, in_=shift_d.ap())
        wneg_sb = consts.tile([128, 2, 3, 64], bf16, tag="wneg")
        nc.sync.dma_start(out=wneg_sb, in_=wneg.ap())

        # ================= Phase A: M1 (chunked loads) =================
        xcT_sb = big.tile([128, 32, 256], bf16, tag="big8k", name="xcT_sb")
        xr_sb = big.tile([128, 32, 256], bf16, tag="big8k", name="xr_sb")
        xcT_r = xcT.ap().rearrange("(t p) c -> p t c", p=128)
        xr_r = xr.ap().rearrange("(t p) c -> p t c", p=128)
        for cch in range(4):
            sl = slice(8 * cch, 8 * cch + 8)
            nc.sync.dma_start(out=xcT_sb[:, sl, :], in_=xcT_r[:, sl, :])
            nc.sync.dma_start(out=xr_sb[:, sl, :], in_=xr_r[:, sl, :])

        m1ps = [psa(), psa()]
        for cch in range(4):
            for t in range(8 * cch, 8 * cch + 8):
                for mc in range(2):
                    nc.tensor.matmul(
                        m1ps[mc][:, 0:256],
                        lhsT=xcT_sb[:, t, 128 * mc:128 * mc + 128],
                        rhs=xr_sb[:, t, :],
                        start=(t == 0), stop=(t == 31))
        for mc in range(2):
            m1e = awork.tile([128, 256], f32, tag="m1e", bufs=2)
            nc.vector.tensor_copy(out=m1e, in_=m1ps[mc][:, 0:256])
            nc.sync.dma_start(out=cc1_in.ap()[128 * mc:128 * mc + 128, :],
                              in_=m1e)
        nc.gpsimd.collective_compute(
            "AllReduce", ALU.add, replica_groups=G8,
            ins=[cc1_in.ap()], outs=[cc1_out.ap()])

        # ================= conv helper (rect taps, +2-row top pad) =========
        # xp tiles are [128, 130, 128] (ch-half, 2+ih, iw); rows 0-1 zero.
        # dj==0 taps read col -1 == previous row col 127 (zero row for oh=0,
        # di=0; real data otherwise) -- corrected by negated-weight matmuls.
        def conv_mms(xpads, octiles, act_func, yout):
            import concourse.bass as bass
            for j in octiles:
                ps = psbf()
                first = True
                for k in range(2):
                    xa = xpads[k][:, :, :]
                    pstep = xa.ap[0][0]
                    for t in (4, 0, 1, 2, 3, 5, 6, 7, 8):
                        di, dj = t // 3, t % 3
                        r0 = 16 * j + di + 1
                        rhs = bass.AP(
                            tensor=xa.tensor,
                            offset=xa.offset + 128 * r0 + dj - 1,
                            ap=[[pstep, 128], [256, 8], [2, 64]])
                        nc.tensor.matmul(
                            ps[0:64, :].rearrange("p (a b) -> p a b", a=8),
                            lhsT=wt_sb[:, k, t, :], rhs=rhs,
                            start=first, stop=False)
                        first = False
                    # left-edge corrections (dj==0 taps wrongly read col -1)
                    for di in range(3):
                        r0 = 16 * j + di
                        rhs = bass.AP(
                            tensor=xa.tensor,
                            offset=xa.offset + 128 * r0 + 127,
                            ap=[[pstep, 128], [256, 8]])
                        nc.tensor.matmul(
                            ps[0:64, 0:449:64], lhsT=wneg_sb[:, k, di, :],
                            rhs=rhs, start=False, stop=(k == 1 and di == 2))
                nc.scalar.activation(
                    out=yout[:, 8 * j: 8 * j + 8, :],
                    in_=ps[0:64, :].rearrange("p (a b) -> p a b", a=8),
                    func=act_func, bias=shift_sb, scale=scale_sb)

        # ---- conv(x) part 1 (octiles 0,1) — fills the AR1 wait ----
        xpx = []
        for k in range(2):
            xp = convin.tile([128, 130, 128], bf16, tag="convin")
            nc.vector.memset(xp[:, 0:2, :], 0.0)
            for par in range(2):
                nc.sync.dma_start(
                    out=xp[64 * par:64 * par + 64, 2:130, :],
                    in_=xb.ap()[128 * k + par:128 * k + 128:2, :, :])
            xpx.append(xp)
        # X halves (contraction rows of Xc) + own-batch block
        X01 = [xpool.tile([128, 8192], bf16, tag="xpool", name=f"X{i}")
               for i in range(2)]
        for i in range(2):
            nc.sync.dma_start(out=X01[i], in_=xcm2.ap()[:, i, :])
        xblk_sb = xpool.tile([128, 8192], bf16, tag="xpool", name="xblk")
        nc.sync.dma_start(out=xblk_sb, in_=xblk.ap())

        y_sb = convout.tile([64, 64, 64], bf16, tag="convout", name="y")
        conv_mms(xpx, (0, 1), AF.Identity, y_sb)

        # ================= Phase A2: softmax(a) + aTo ===================
        a_bf = []
        rinv = []
        for ch in range(2):
            a_raw = awork.tile([128, 256], f32, tag="a_raw")
            nc.sync.dma_start(out=a_raw,
                              in_=cc1_out.ap()[128 * ch:128 * ch + 128, :])
            negmax = awork.tile([128, 1], f32, tag="negmax")
            nc.vector.tensor_reduce(out=negmax, in_=a_raw, axis=AX.X,
                                    op=ALU.max, negate=True)
            a_e = awork.tile([128, 256], bf16, tag="a_e", bufs=2)
            asum = awork.tile([128, 1], f32, tag="asum")
            nc.scalar.activation(out=a_e, in_=a_raw, func=AF.Exp,
                                 bias=negmax, scale=1.0, accum_out=asum)
            ri = awork.tile([128, 1], f32, tag="ri", bufs=2)
            nc.vector.reciprocal(out=ri, in_=asum)
            a_bf.append(a_e)
            rinv.append(ri)

        aTo = []
        for c2c in range(2):
            ps = psa()
            for c1c in range(2):
                nc.tensor.matmul(ps[:, 0:128],
                                 lhsT=a_bf[c1c][:, 128 * c2c:128 * c2c + 128],
                                 rhs=sel_sb[:, c1c, :],
                                 start=(c1c == 0), stop=(c1c == 1))
            t_ = awork.tile([128, 128], bf16, tag="aTo", bufs=2)
            nc.vector.tensor_copy(out=t_, in_=ps[:, 0:128])
            aTo.append(t_)
        sel_f = awork.tile([128, 2, 128], f32, tag="sel_f")
        nc.vector.tensor_copy(out=sel_f, in_=sel_sb)
        ps = psa()
        for c1c in range(2):
            nc.tensor.matmul(ps[:, 0:1], lhsT=sel_f[:, c1c, :],
                             rhs=rinv[c1c], start=(c1c == 0), stop=(c1c == 1))
        rinv_own = awork.tile([128, 1], f32, tag="rinv_own")
        nc.vector.tensor_copy(out=rinv_own, in_=ps[:, 0:1])

        # ================= Phase B: M2 -> E = exp(s) =================
        E_h = [big.tile([128, 4096], bf16, tag="big8k", name=f"E{h}")
               for h in range(2)]
        partials = awork.tile([128, 16], f32, tag="partials")
        for nci in range(16):
            ps = psa()
            for bp in range(2):
                nc.tensor.matmul(ps,
                                 lhsT=aTo[bp],
                                 rhs=X01[bp][:, 512 * nci:512 * nci + 512],
                                 start=(bp == 0), stop=(bp == 1))
            nc.scalar.activation(
                out=E_h[nci // 8][:, 512 * (nci % 8): 512 * (nci % 8) + 512],
                in_=ps, func=AF.Exp, scale=rinv_own,
                accum_out=partials[:, nci:nci + 1])
        sloc = awork.tile([128, 1], f32, tag="sloc")
        nc.vector.tensor_reduce(out=sloc, in_=partials, axis=AX.X, op=ALU.add)
        nc.sync.dma_start(out=cc2_in.ap(), in_=sloc)
        nc.gpsimd.collective_compute(
            "AllReduce", ALU.add, replica_groups=G4,
            ins=[cc2_in.ap()], outs=[cc2_out.ap()])
        psA_cm.__exit__(None, None, None)
        psT = ctx.enter_context(tc.tile_pool(name="psT", bufs=4, space="PSUM"))

        def pst_f():
            return psT.tile([128, 512], f32, tag="psT", name="psT_f")

        def pst_b():
            return psT.tile([128, 512], bf16, tag="psT", name="psT_b")

        gsum = awork.tile([128, 1], f32, tag="gsum")
        nc.sync.dma_start(out=gsum, in_=cc2_out.ap())
        corr = awork.tile([128, 1], f32, tag="corr")
        nc.vector.reciprocal(out=corr, in_=gsum)

        # ---- conv(x) part 2 (octiles 2..7) ----
        conv_mms(xpx, range(2, 8), AF.Identity, y_sb)

        # ================= z-chain (2 halves) + split AllGather =========
        cc_h = [(cca_in, cca_out), (ccb_in, ccb_out)]
        z_h = []
        for h in range(2):
            zh = big.tile([128, 4096], bf16, tag="big8k", name=f"z{h}")
            nc.scalar.activation(out=zh, in_=E_h[h], func=AF.Exp, scale=corr)
            z_h.append(zh)
        for h in range(2):
            zh = z_h[h]
            zv = zh.rearrange("p (h w) -> p h w", w=128)
            zs = awork.tile([128, 32], f32, tag="zs", bufs=2)
            nc.vector.tensor_reduce(out=zs, in_=zv, axis=AX.X, op=ALU.add)
            zr = awork.tile([128, 32], f32, tag="zr", bufs=2)
            nc.vector.reciprocal(out=zr, in_=zs)
            nc.vector.tensor_tensor(out=zv, in0=zv, in1=_bcast_last(zr, 128),
                                    op=ALU.mult)
            ciah = big.tile([128, 4096], bf16, tag="big8k", name=f"cia{h}")
            nc.vector.tensor_tensor(out=ciah, in0=zh,
                                    in1=xblk_sb[:, 4096 * h:4096 * h + 4096],
                                    op=ALU.add)
            nc.sync.dma_start(out=cc_h[h][0].ap(), in_=ciah)
            nc.gpsimd.collective_compute(
                "AllGather", ALU.bypass, replica_groups=G4,
                ins=[cc_h[h][0].ap()], outs=[cc_h[h][1].ap()])

        # ================= transposes of y (dup halves) =================
        # brT_dup/x43T_dup: [p, oh, c] with p = w (0-63) and w+64 (dup copy)
        brT_dup = xpool.tile([128, 64, 64], bf16, tag="xpool", name="brT")
        x43T_dup = xpool.tile([128, 64, 64], bf16, tag="xpool", name="x43T")
        id64a = id128[0:64, 0:64]
        for chunk in range(8):
            ps = pst_b()
            for i in range(8):
                oh = 8 * chunk + i
                nc.tensor.transpose(ps[0:64, 64 * i:64 * i + 64],
                                    in_=y_sb[:, oh, :], identity=id64a,
                                    tile_position=(0, 0))
                nc.tensor.transpose(ps[64:128, 64 * i:64 * i + 64],
                                    in_=y_sb[:, oh, :], identity=id64a,
                                    tile_position=(0, 64))
            pv = ps.rearrange("p (a b) -> p a b", a=8)
            csl = slice(8 * chunk, 8 * chunk + 8)
            nc.scalar.activation(out=brT_dup[:, csl, :], in_=pv, func=AF.Relu)
            tmpl = attn.tile([128, 8, 64], bf16, tag="tmpl", bufs=1)
            nc.vector.tensor_scalar_mul(tmpl, pv, 0.2)
            tmpl2 = attn.tile([128, 8, 64], bf16, tag="tmpl2", bufs=1)
            nc.vector.tensor_tensor(out=tmpl2, in0=pv, in1=tmpl, op=ALU.max)
            nc.scalar.activation(out=x43T_dup[:, csl, :], in_=tmpl2,
                                 func=AF.Sigmoid)

        # ================= T1: att2 + a2T (during AllGather) ============
        a2T_all = awork.tile([128, 8, 4, 64], bf16, tag="a2T_all")
        id64b = id128[64:128, 64:128]
        for g in range(8):
            ps2 = pst_f()
            for m in range(4):
                cA, cB = 8 * g + 2 * m, 8 * g + 2 * m + 1
                msl = slice(64 * m, 64 * m + 64)
                nc.tensor.matmul(ps2[0:64, msl], lhsT=brT_dup[0:64, :, cA],
                                 rhs=x43T_dup[0:64, :, cA], start=True,
                                 stop=True, tile_position=(0, 0))
                nc.tensor.matmul(ps2[64:128, msl], lhsT=brT_dup[64:128, :, cB],
                                 rhs=x43T_dup[64:128, :, cB], start=True,
                                 stop=True, tile_position=(64, 64))
            v2 = ps2[:, 0:256].rearrange("p (a b) -> p a b", a=4)
            e2 = attn.tile([128, 4, 64], bf16, tag="e2")
            nc.scalar.activation(out=e2, in_=v2, func=AF.Exp)
            s2 = awork.tile([128, 4], f32, tag="s2", bufs=2)
            nc.vector.tensor_reduce(out=s2, in_=e2, axis=AX.X, op=ALU.add)
            r2 = awork.tile([128, 4], f32, tag="r2", bufs=2)
            nc.vector.reciprocal(out=r2, in_=s2)
            att2sm = attn.tile([128, 4, 64], bf16, tag="att2sm")
            nc.vector.tensor_tensor(out=att2sm, in0=e2, in1=_bcast_last(r2, 64),
                                    op=ALU.mult)
            psTt = pst_b()
            for m in range(4):
                msl = slice(64 * m, 64 * m + 64)
                nc.tensor.transpose(psTt[0:64, msl], in_=att2sm[0:64, m, :],
                                    identity=id64a, tile_position=(0, 0))
                nc.tensor.transpose(psTt[64:128, msl], in_=att2sm[64:128, m, :],
                                    identity=id64b, tile_position=(64, 64))
            nc.vector.tensor_copy(
                out=a2T_all[:, g, :, :],
                in_=psTt[:, 0:256].rearrange("p (a b) -> p a b", a=4))

        # ================= conv(cia) =================
        xpc = []
        for k in range(2):
            xp = convin.tile([128, 130, 128], bf16, tag="convin")
            nc.vector.memset(xp[:, 0:2, :], 0.0)
            for half in range(2):
                src_t = cc_h[half][1]
                for par in range(2):
                    for hh in range(2):
                        src = src_t.ap()[2 * par + hh,
                                         64 * k:64 * k + 64, :] \
                            .rearrange("p (h w) -> p h w", w=128)
                        r0 = 2 + 64 * hh + 32 * half
                        dst = xp[64 * par:64 * par + 64, r0:r0 + 32, :]
                        nc.sync.dma_start(out=dst, in_=src)
            xpc.append(xp)
        x12_sb = convout.tile([64, 64, 64], bf16, tag="convout", name="x12")
        conv_mms(xpc, (0, 1, 5), AF.Relu, x12_sb)
        conv_mms(xpc, (2, 3, 4, 6, 7), AF.Relu, x12_sb)

        # ---- x12T_dup transposes ----
        x12T_dup = xpool.tile([128, 64, 64], bf16, tag="xpool", name="x12T")
        for chunk in range(8):
            ps = pst_b()
            for i in range(8):
                oh = 8 * chunk + i
                nc.tensor.transpose(ps[0:64, 64 * i:64 * i + 64],
                                    in_=x12_sb[:, oh, :], identity=id64a,
                                    tile_position=(0, 0))
                nc.tensor.transpose(ps[64:128, 64 * i:64 * i + 64],
                                    in_=x12_sb[:, oh, :], identity=id64a,
                                    tile_position=(0, 64))
            nc.vector.tensor_copy(
                out=x12T_dup[:, 8 * chunk:8 * chunk + 8, :],
                in_=ps.rearrange("p (a b) -> p a b", a=8))

        # ================= T2: att1, x3_3, S, upsample =================
        for g in range(8):
            def qmm(psd, lt, rh):
                for m in range(4):
                    cA, cB = 8 * g + 2 * m, 8 * g + 2 * m + 1
                    msl = slice(64 * m, 64 * m + 64)
                    nc.tensor.matmul(psd[0:64, msl], lhsT=lt[0:64, :, cA],
                                     rhs=rh(0, m, cA), start=True, stop=True,
                                     tile_position=(0, 0))
                    nc.tensor.matmul(psd[64:128, msl], lhsT=lt[64:128, :, cB],
                                     rhs=rh(64, m, cB), start=True, stop=True,
                                     tile_position=(64, 64))

            # --- att1 (max-shifted softmax) ---
            ps1 = pst_f()
            qmm(ps1, x12T_dup,
                lambda o, m, c: brT_dup[o:o + 64, :, c])
            v1 = ps1[:, 0:256].rearrange("p (a b) -> p a b", a=4)
            nm1 = awork.tile([128, 4], f32, tag="nm1", bufs=2)
            nc.vector.tensor_reduce(out=nm1, in_=v1, axis=AX.X, op=ALU.max,
                                    negate=True)
            lg1 = attn.tile([128, 4, 64], bf16, tag="lg1", bufs=1)
            nc.vector.tensor_tensor(out=lg1, in0=v1, in1=_bcast_last(nm1, 64),
                                    op=ALU.add)
            e1 = attn.tile([128, 4, 64], bf16, tag="e1")
            nc.scalar.activation(out=e1, in_=lg1, func=AF.Exp)
            s1 = awork.tile([128, 4], f32, tag="s1", bufs=2)
            nc.vector.tensor_reduce(out=s1, in_=e1, axis=AX.X, op=ALU.add)
            r1 = awork.tile([128, 4], f32, tag="r1", bufs=2)
            nc.vector.reciprocal(out=r1, in_=s1)
            att1sm = attn.tile([128, 4, 64], bf16, tag="att1sm")
            nc.vector.tensor_tensor(out=att1sm, in0=e1, in1=_bcast_last(r1, 64),
                                    op=ALU.mult)

            # --- x3_3 (shift-free) ---
            ps3 = pst_f()
            qmm(ps3, x12T_dup,
                lambda o, m, c: a2T_all[o:o + 64, g, m, :])
            e3 = attn.tile([128, 4, 64], bf16, tag="e3")
            nc.scalar.activation(out=e3,
                                 in_=ps3[:, 0:256].rearrange("p (a b) -> p a b", a=4),
                                 func=AF.Exp)
            s3 = awork.tile([128, 4], f32, tag="s3", bufs=2)
            nc.vector.tensor_reduce(out=s3, in_=e3, axis=AX.X, op=ALU.add)
            r3 = awork.tile([128, 4], f32, tag="r3", bufs=2)
            nc.vector.reciprocal(out=r3, in_=s3)
            x33sm = attn.tile([128, 4, 64], bf16, tag="x33sm")
            nc.vector.tensor_tensor(out=x33sm, in0=e3, in1=_bcast_last(r3, 64),
                                    op=ALU.mult)

            # --- branch natural layout via transpose ---
            psB2 = pst_b()
            for m in range(4):
                cA, cB = 8 * g + 2 * m, 8 * g + 2 * m + 1
                msl = slice(64 * m, 64 * m + 64)
                nc.tensor.transpose(psB2[0:64, msl], in_=brT_dup[0:64, :, cA],
                                    identity=id64a, tile_position=(0, 0))
                nc.tensor.transpose(psB2[64:128, msl],
                                    in_=brT_dup[64:128, :, cB],
                                    identity=id64b, tile_position=(64, 64))

            # --- S = relu(x33sm + att1sm + br_nat) ---
            t2 = attn.tile([128, 4, 64], bf16, tag="t2", bufs=1)
            nc.vector.tensor_tensor(out=t2, in0=x33sm, in1=att1sm, op=ALU.add)
            t3 = attn.tile([128, 4, 64], bf16, tag="t3", bufs=1)
            nc.vector.tensor_tensor(
                out=t3, in0=t2,
                in1=psB2[:, 0:256].rearrange("p (a b) -> p a b", a=4), op=ALU.add)
            S_sb = attn.tile([128, 4, 64], bf16, tag="S", bufs=1)
            nc.vector.tensor_scalar_max(S_sb, t3, 0.0)

            # --- upsample: W1 = R @ S (row-tiled halves) ---
            psW = pst_f()
            nc.tensor.matmul(psW[:, 0:256], lhsT=rt_sb[0:64, :],
                             rhs=S_sb[0:64, :, :], start=True, stop=True)
            nc.tensor.matmul(psW[0:64, 256:512], lhsT=rt_sb[64:128, 0:64],
                             rhs=S_sb[64:128, :, :], start=True, stop=True,
                             tile_position=(64, 0))
            nc.tensor.matmul(psW[64:128, 256:512], lhsT=rt_sb[64:128, 64:128],
                             rhs=S_sb[64:128, :, :], start=True, stop=True,
                             tile_position=(64, 64))
            W1 = attn.tile([128, 8, 64], bf16, tag="W1")
            nc.vector.tensor_copy(out=W1,
                                  in_=psW.rearrange("p (a b) -> p a b", a=8))
            # --- W1T (col-tiled transposes of [128,64] slices) ---
            psT2 = psT.tile([128, 4, 128], bf16, tag="psT", name="psT2")
            for m in range(4):
                nc.tensor.transpose(psT2[0:64, m, :], in_=W1[:, m, :],
                                    identity=id128, tile_position=(0, 0))
                nc.tensor.transpose(psT2[64:128, m, :], in_=W1[:, 4 + m, :],
                                    identity=id128, tile_position=(0, 64))
            W1T = attn.tile([128, 4, 128], bf16, tag="W1T")
            nc.vector.tensor_copy(out=W1T, in_=psT2)
            # --- U = W1 @ R^T (row-tiled pairs) ---
            psUa = psT.tile([128, 4, 128], f32, tag="psT", name="psUa")
            psUb = psT.tile([128, 4, 128], f32, tag="psT", name="psUb")
            for m in range(4):
                nc.tensor.matmul(psUa[:, m, :], lhsT=W1T[0:64, m, :],
                                 rhs=rt_sb[0:64, :], start=True, stop=True)
                nc.tensor.matmul(psUb[0:64, m, :],
                                 lhsT=W1T[64:128, m, 0:64],
                                 rhs=rt_sb[64:128, :], start=True, stop=True,
                                 tile_position=(64, 0))
                nc.tensor.matmul(psUb[64:128, m, :],
                                 lhsT=W1T[64:128, m, 64:128],
                                 rhs=rt_sb[64:128, :], start=True, stop=True,
                                 tile_position=(64, 64))
            uA = attn.tile([128, 4, 128], bf16, tag="uA")
            nc.vector.tensor_copy(out=uA, in_=psUa)
            uB = attn.tile([128, 4, 128], bf16, tag="uB")
            nc.vector.tensor_copy(out=uB, in_=psUb)
            nc.sync.dma_start(
                out=out_sh.ap()[8 * g:8 * g + 8:2, :, :]
                .rearrange("c h w -> h c w"), in_=uA)
            nc.sync.dma_start(
                out=out_sh.ap()[8 * g + 1:8 * g + 8:2, :, :]
                .rearrange("c h w -> h c w"), in_=uB)

    nc.finalize()
    return nc


def host_prep(inputs):
    """Build per-core in_maps (host does only slicing/transpose/cast)."""
    bfd = ml_dtypes.bfloat16
    x = np.asarray(inputs["x"], np.float32)
    conv_w = np.asarray(inputs["conv_w"], np.float32)
    conv_b = np.asarray(inputs["conv_b"], np.float32)
    g = np.asarray(inputs["bn_gamma"], np.float32)
    be = np.asarray(inputs["bn_beta"], np.float32)
    mu = np.asarray(inputs["bn_mean"], np.float32)
    var = np.asarray(inputs["bn_var"], np.float32)

    scale = (g / np.sqrt(var + EPS)).astype(np.float32)
    shift = ((conv_b - mu) * scale + be).astype(np.float32)
    Xc = x.reshape(C, -1)
    Xr = x.reshape(-1, C)
    RT = np.ascontiguousarray(_resize_mat(H, HO).T).astype(bfd)
    RT2 = np.ascontiguousarray(np.vstack([RT, RT]))
    wtr = conv_w.transpose(1, 0, 2, 3)  # (ci, co, 3, 3)

    in_maps = []
    for p in range(N_CORES):
        b, q = p // 4, p % 4
        xcT_h = np.ascontiguousarray(Xc[:, 4096 * p:4096 * (p + 1)].T).astype(bfd)
        xr_h = np.ascontiguousarray(Xr[4096 * p:4096 * (p + 1), :]).astype(bfd)
        xcm2_h = np.ascontiguousarray(
            Xc[:, 8192 * q:8192 * (q + 1)].reshape(2, 128, 8192)
            .transpose(1, 0, 2)).astype(bfd)
        xblk_h = np.ascontiguousarray(xcm2_h[:, b, :])
        sel_h = np.zeros((128, 2, 128), np.float32)
        sel_h[np.arange(128), b, np.arange(128)] = 1.0
        cperm = np.concatenate([np.arange(0, 128, 2), np.arange(1, 128, 2)])
        wt_h = np.ascontiguousarray(
            wtr[:, 64 * q:64 * q + 64].reshape(2, 128, 64, 9)
            .transpose(1, 0, 3, 2)[cperm]).astype(bfd)
        wneg_h = np.ascontiguousarray(
            -wtr[:, 64 * q:64 * q + 64, :, 0].reshape(2, 128, 64, 3)
            .transpose(1, 0, 3, 2)[cperm]).astype(bfd)
        in_maps.append({
            "xb": x[b].astype(bfd),
            "wneg": wneg_h,
            "xcT": xcT_h,
            "xr": xr_h,
            "xcm2": xcm2_h,
            "xblk": xblk_h,
            "sel": sel_h.astype(bfd),
            "wt": wt_h,
            "scale": scale[64 * q:64 * q + 64].reshape(64, 1),
            "shift": shift[64 * q:64 * q + 64].reshape(64, 1),
            "rt": RT2,
        })
    return in_maps


_NC = None


def _get_nc():
    global _NC
    if _NC is None:
        _NC = build_nc()
    return _NC


def run_device(inputs, trace=False):
    from concourse.bass_utils import run_bass_kernel_spmd
    nc = _get_nc()
    in_maps = host_prep(inputs)
    try:
        res = run_bass_kernel_spmd(nc, in_maps, list(range(N_CORES)),
                                   trace=trace)
    except (ImportError, ModuleNotFoundError):
        # NTFF profiling hook unavailable in this environment
        res = run_bass_kernel_spmd(nc, in_maps, list(range(N_CORES)),
                                   trace=False)
    out = np.zeros((B, C, H, W), np.float32)
    for p in range(N_CORES):
        b, q = p // 4, p % 4
        out[b, 64 * q:64 * q + 64] = res.results[p]["out_sh"].astype(np.float32)
    return out, res


def kernel(**inputs):
    out, _ = run_device(inputs, trace=False)
    return out


# revision 19
# speedup vs baseline: 9436.5023x; 9436.5023x over previous
"""Self-contained Bass/Tile SPMD kernel for nn_AIA_1_56049323213170 (8 NeuronCores).

Pipeline (B=2, C=256, H=W=128), all heavy math on-device in bf16/f32-psum:
  M1 = Xc @ Xr (CxC, contraction sharded 8-way + AllReduce)
  a  = rowsoftmax(M1)  (redundant per core, unnormalized + row-recip trick)
  s  = a @ Xc          (sharded: rows by batch, cols by quarter -> (128, 8192))
  rowsoftmax(s) needs only a global row-sum (|s|<=5.5 -> shift-free exp):
       AllReduce of per-core row sums within each batch group of 4 cores
  cia = x + softmax_W(rowsoftmax(s))   (W-softmax local, shift-free)
  AllGather cia within batch group (split in 2 halves for pipelining)
  y   = BN(conv3x3s2(x)+b), x1_2 = relu(BN(conv3x3s2(cia)+b))  (64 out-ch/core)
  branch = relu(y); x4_3 = sigmoid(lrelu(y,.2))
  att1 = rowsoftmax(x1_2 @ branch^T); att2 = rowsoftmax(branch @ x4_3^T)
  x3_3 = rowsoftmax(x1_2 @ att2^T)  (att2/x3_3 shift-free)
  out  = bilinear_up2(relu(x3_3 + att1 + branch)) via R @ S @ R^T

v2: overlap-oriented schedule (conv-x + att2 prep run during the collective
chain), unpadded conv inputs with ragged edge taps (contiguous DMA), and a
PSUM-quadrant-packed attention tail (two channel-pairs per instruction).

Core p: b = p//4 (batch), q = p%4 (quarter; parity=q//2, h-half=q%2).
Each core returns out[b, 64q:64q+64] as bf16; host assembles f32.
"""
import numpy as np
import ml_dtypes

N_CORES = 8
B, C, H, W = 2, 256, 128, 128
HO = WO = 64
EPS = 1e-5


def _resize_mat(n_out, n_in):
    R = np.zeros((n_out, n_in), np.float32)
    scale = n_in / n_out
    for i in range(n_out):
        src = (i + 0.5) * scale - 0.5
        i0 = int(np.floor(src))
        frac = src - i0
        lo = min(max(i0, 0), n_in - 1)
        hi = min(max(i0 + 1, 0), n_in - 1)
        R[i, lo] += 1.0 - frac
        R[i, hi] += frac
    return R


def _ap_of(t):
    import concourse.bass as bass
    if isinstance(t, bass.AP):
        return t
    return t.ap()


def _bcast_last(t, n):
    """AP of tile t broadcast with a 0-step innermost dim of size n."""
    import concourse.bass as bass
    base = _ap_of(t)
    return bass.AP(tensor=base.tensor, offset=base.offset,
                   ap=[list(d) for d in base.ap] + [[0, n]])


def build_nc():
    from contextlib import ExitStack
    import concourse.bass as bass
    import concourse.mybir as mybir
    import concourse.tile as tile
    from concourse import bacc
    from concourse.masks import make_identity

    f32 = mybir.dt.float32
    bf16 = mybir.dt.bfloat16
    AF = mybir.ActivationFunctionType
    AX = mybir.AxisListType
    ALU = mybir.AluOpType

    nc = bacc.Bacc("TRN2", target_bir_lowering=False, debug=False,
                   num_devices=N_CORES)

    # ---- I/O ----
    xb = nc.dram_tensor("xb", [C, H, W], bf16, kind="ExternalInput")
    xcT = nc.dram_tensor("xcT", [4096, 256], bf16, kind="ExternalInput")
    xr = nc.dram_tensor("xr", [4096, 256], bf16, kind="ExternalInput")
    xcm2 = nc.dram_tensor("xcm2", [128, 2, 8192], bf16, kind="ExternalInput")
    xblk = nc.dram_tensor("xblk", [128, 8192], bf16, kind="ExternalInput")
    sel = nc.dram_tensor("sel", [128, 2, 128], bf16, kind="ExternalInput")
    wt = nc.dram_tensor("wt", [128, 2, 9, 64], bf16, kind="ExternalInput")
    scale_d = nc.dram_tensor("scale", [64, 1], f32, kind="ExternalInput")
    shift_d = nc.dram_tensor("shift", [64, 1], f32, kind="ExternalInput")
    rt_d = nc.dram_tensor("rt", [128, 128], bf16, kind="ExternalInput")
    wneg = nc.dram_tensor("wneg", [128, 2, 3, 64], bf16, kind="ExternalInput")
    out_sh = nc.dram_tensor("out_sh", [64, H, W], bf16, kind="ExternalOutput")

    # ---- collective scratch ----
    cc1_in = nc.dram_tensor("cc1_in", [256, 256], f32)
    cc1_out = nc.dram_tensor("cc1_out", [256, 256], f32, addr_space="Shared")
    cc2_in = nc.dram_tensor("cc2_in", [128, 1], f32)
    cc2_out = nc.dram_tensor("cc2_out", [128, 1], f32)
    cca_in = nc.dram_tensor("cca_in", [128, 4096], bf16)
    cca_out = nc.dram_tensor("cca_out", [4, 128, 4096], bf16)
    ccb_in = nc.dram_tensor("ccb_in", [128, 4096], bf16)
    ccb_out = nc.dram_tensor("ccb_out", [4, 128, 4096], bf16)
    G8 = [list(range(8))]
    G4 = [[0, 1, 2, 3], [4, 5, 6, 7]]

    with tile.TileContext(nc) as tc, ExitStack() as ctx:
        consts = ctx.enter_context(tc.tile_pool(name="consts", bufs=1))
        awork = ctx.enter_context(tc.tile_pool(name="awork", bufs=1))
        big = ctx.enter_context(tc.tile_pool(name="big", bufs=3))
        xpool = ctx.enter_context(tc.tile_pool(name="xpool", bufs=3))
        convin = ctx.enter_context(tc.tile_pool(name="convin", bufs=2))
        convout = ctx.enter_context(tc.tile_pool(name="convout", bufs=1))
        attn = ctx.enter_context(tc.tile_pool(name="attn", bufs=2))
        psB_cm = tc.tile_pool(name="psB", bufs=4, space="PSUM")
        psB = psB_cm.__enter__()
        psA_cm = tc.tile_pool(name="psA", bufs=3, space="PSUM")
        psA = psA_cm.__enter__()

        def psa():
            return psA.tile([128, 512], f32, tag="psA", name="psA_t")

        def psbf():
            return psB.tile([128, 512], f32, tag="psB", name="psB_t")

        # ================= constants =================
        id128 = consts.tile([128, 128], bf16, tag="id128")
        make_identity(nc, id128)
        rt_sb = consts.tile([128, 128], bf16, tag="rt")
        nc.sync.dma_start(out=rt_sb, in_=rt_d.ap())
        sel_sb = consts.tile([128, 2, 128], bf16, tag="sel")
        nc.sync.dma_start(out=sel_sb, in_=sel.ap())
        wt_sb = consts.tile([128, 2, 9, 64], bf16, tag="wt")
        nc.sync.dma_start(out=wt_sb, in_=wt.ap())
        scale_sb = consts.tile([64, 1], f32, tag="scale")
        nc.sync.dma_start(out=scale_sb, in_=scale_d.ap())
        shift_sb = consts.tile([64, 1], f32, tag="shift")
        nc.sync.dma_start(out=shift_sb, in_=shift_d.ap())
        wneg_sb = consts.tile([128, 2, 3, 64], bf16, tag="wneg")
        nc.sync.dma_start(out=wneg_sb, in_=wneg.ap())

        # ================= Phase A: M1 (chunked loads) =================
        xcT_sb = big.tile([128, 32, 256], bf16, tag="big8k", name="xcT_sb")
        xr_sb = big.tile([128, 32, 256], bf16, tag="big8k", name="xr_sb")
        xcT_r = xcT.ap().rearrange("(t p) c -> p t c", p=128)
        xr_r = xr.ap().rearrange("(t p) c -> p t c", p=128)
        for cch in range(4):
            sl = slice(8 * cch, 8 * cch + 8)
            nc.sync.dma_start(out=xcT_sb[:, sl, :], in_=xcT_r[:, sl, :])
            nc.sync.dma_start(out=xr_sb[:, sl, :], in_=xr_r[:, sl, :])

        m1ps = [psa(), psa()]
        for cch in range(4):
            for t in range(8 * cch, 8 * cch + 8):
                for mc in range(2):
                    nc.tensor.matmul(
                        m1ps[mc][:, 0:256],
                        lhsT=xcT_sb[:, t, 128 * mc:128 * mc + 128],
                        rhs=xr_sb[:, t, :],
                        start=(t == 0), stop=(t == 31))
        for mc in range(2):
            m1e = awork.tile([128, 256], f32, tag="m1e", bufs=2)
            nc.vector.tensor_copy(out=m1e, in_=m1ps[mc][:, 0:256])
            nc.sync.dma_start(out=cc1_in.ap()[128 * mc:128 * mc + 128, :],
                              in_=m1e)
        nc.gpsimd.collective_compute(
            "AllReduce", ALU.add, replica_groups=G8,
            ins=[cc1_in.ap()], outs=[cc1_out.ap()])

        # ================= conv helper (rect taps, +2-row top pad) =========
        # xp tiles are [128, 130, 128] (ch-half, 2+ih, iw); rows 0-1 zero.
        # dj==0 taps read col -1 == previous row col 127 (zero row for oh=0,
        # di=0; real data otherwise) -- corrected by negated-weight matmuls.
        def conv_mms(xpads, octiles, act_func, yout):
            import concourse.bass as bass
            for j in octiles:
                ps = psbf()
                first = True
                for k in range(2):
                    xa = xpads[k][:, :, :]
                    pstep = xa.ap[0][0]
                    for t in (4, 0, 1, 2, 3, 5, 6, 7, 8):
                        di, dj = t // 3, t % 3
                        r0 = 16 * j + di + 1
                        rhs = bass.AP(
                            tensor=xa.tensor,
                            offset=xa.offset + 128 * r0 + dj - 1,
                            ap=[[pstep, 128], [256, 8], [2, 64]])
                        nc.tensor.matmul(
                            ps[0:64, :].rearrange("p (a b) -> p a b", a=8),
                            lhsT=wt_sb[:, k, t, :], rhs=rhs,
                            start=first, stop=False)
                        first = False
                    # left-edge corrections (dj==0 taps wrongly read col -1)
                    for di in range(3):
                        r0 = 16 * j + di
                        rhs = bass.AP(
                            tensor=xa.tensor,
                            offset=xa.offset + 128 * r0 + 127,
                            ap=[[pstep, 128], [256, 8]])
                        nc.tensor.matmul(
                            ps[0:64, 0:449:64], lhsT=wneg_sb[:, k, di, :],
                            rhs=rhs, start=False, stop=(k == 1 and di == 2))
                nc.scalar.activation(
                    out=yout[:, 8 * j: 8 * j + 8, :],
                    in_=ps[0:64, :].rearrange("p (a b) -> p a b", a=8),
                    func=act_func, bias=shift_sb, scale=scale_sb)

        # ---- conv(x) part 1 (octiles 0,1) — fills the AR1 wait ----
        xpx = []
        for k in range(2):
            xp = convin.tile([128, 130, 128], bf16, tag="convin")
            nc.vector.memset(xp[:, 0:2, :], 0.0)
            for par in range(2):
                nc.sync.dma_start(
                    out=xp[64 * par:64 * par + 64, 2:66, :],
                    in_=xb.ap()[128 * k + par:128 * k + 128:2, 0:64, :])
            for par in range(2):
                nc.sync.dma_start(
                    out=xp[64 * par:64 * par + 64, 66:130, :],
                    in_=xb.ap()[128 * k + par:128 * k + 128:2, 64:128, :])
            xpx.append(xp)
        # X halves (contraction rows of Xc) + own-batch block
        X01 = [xpool.tile([128, 8192], bf16, tag="xpool", name=f"X{i}")
               for i in range(2)]
        for i in range(2):
            nc.sync.dma_start(out=X01[i], in_=xcm2.ap()[:, i, :])
        xblk_sb = xpool.tile([128, 8192], bf16, tag="xpool", name="xblk")
        nc.sync.dma_start(out=xblk_sb, in_=xblk.ap())

        y_sb = convout.tile([64, 64, 64], bf16, tag="convout", name="y")
        conv_mms(xpx, (0, 1), AF.Identity, y_sb)

        # ================= Phase A2: softmax(a) + aTo ===================
        a_bf = []
        rinv = []
        for ch in range(2):
            a_raw = awork.tile([128, 256], f32, tag="a_raw")
            nc.sync.dma_start(out=a_raw,
                              in_=cc1_out.ap()[128 * ch:128 * ch + 128, :])
            negmax = awork.tile([128, 1], f32, tag="negmax")
            nc.vector.tensor_reduce(out=negmax, in_=a_raw, axis=AX.X,
                                    op=ALU.max, negate=True)
            a_e = awork.tile([128, 256], bf16, tag="a_e", bufs=2)
            asum = awork.tile([128, 1], f32, tag="asum")
            nc.scalar.activation(out=a_e, in_=a_raw, func=AF.Exp,
                                 bias=negmax, scale=1.0, accum_out=asum)
            ri = awork.tile([128, 1], f32, tag="ri", bufs=2)
            nc.vector.reciprocal(out=ri, in_=asum)
            a_bf.append(a_e)
            rinv.append(ri)

        aTo = []
        for c2c in range(2):
            ps = psa()
            for c1c in range(2):
                nc.tensor.matmul(ps[:, 0:128],
                                 lhsT=a_bf[c1c][:, 128 * c2c:128 * c2c + 128],
                                 rhs=sel_sb[:, c1c, :],
                                 start=(c1c == 0), stop=(c1c == 1))
            t_ = awork.tile([128, 128], bf16, tag="aTo", bufs=2)
            nc.vector.tensor_copy(out=t_, in_=ps[:, 0:128])
            aTo.append(t_)
        sel_f = awork.tile([128, 2, 128], f32, tag="sel_f")
        nc.vector.tensor_copy(out=sel_f, in_=sel_sb)
        ps = psa()
        for c1c in range(2):
            nc.tensor.matmul(ps[:, 0:1], lhsT=sel_f[:, c1c, :],
                             rhs=rinv[c1c], start=(c1c == 0), stop=(c1c == 1))
        rinv_own = awork.tile([128, 1], f32, tag="rinv_own")
        nc.vector.tensor_copy(out=rinv_own, in_=ps[:, 0:1])

        # ================= Phase B: M2 -> E = exp(s) =================
        E_h2 = big.tile([128, 8192], bf16, tag="big8k", name="E")
        partials = awork.tile([128, 16], f32, tag="partials")
        for nci in range(16):
            ps = psa()
            for bp in range(2):
                nc.tensor.matmul(ps,
                                 lhsT=aTo[bp],
                                 rhs=X01[bp][:, 512 * nci:512 * nci + 512],
                                 start=(bp == 0), stop=(bp == 1))
            nc.scalar.activation(
                out=E_h2[:, 512 * nci: 512 * nci + 512],
                in_=ps, func=AF.Exp, scale=rinv_own,
                accum_out=partials[:, nci:nci + 1])
        sloc = awork.tile([128, 1], f32, tag="sloc")
        nc.vector.tensor_reduce(out=sloc, in_=partials, axis=AX.X, op=ALU.add)
        nc.sync.dma_start(out=cc2_in.ap(), in_=sloc)
        nc.gpsimd.collective_compute(
            "AllReduce", ALU.add, replica_groups=G4,
            ins=[cc2_in.ap()], outs=[cc2_out.ap()])
        psA_cm.__exit__(None, None, None)
        psT_cm = tc.tile_pool(name="psT", bufs=4, space="PSUM")
        psT = psT_cm.__enter__()

        def pst_f():
            return psT.tile([128, 512], f32, tag="psT", name="psT_f")

        def pst_b():
            return psT.tile([128, 512], bf16, tag="psT", name="psT_b")

        gsum = awork.tile([128, 1], f32, tag="gsum")
        nc.sync.dma_start(out=gsum, in_=cc2_out.ap())
        corr = awork.tile([128, 1], f32, tag="corr")
        nc.vector.reciprocal(out=corr, in_=gsum)

        # ---- conv(x) part 2 (octiles 2..7) ----
        conv_mms(xpx, range(2, 8), AF.Identity, y_sb)

        # ================= z-chain (2 halves) + split AllGather =========
        cc_h = [(cca_in, cca_out), (ccb_in, ccb_out)]
        HSPLIT = 4096
        for h, (c0, c1) in enumerate(((0, HSPLIT), (HSPLIT, 8192))):
            zh = big.tile([128, c1 - c0], bf16, tag="big8k", name=f"z{h}")
            ciah = big.tile([128, c1 - c0], bf16, tag="big8k", name=f"cia{h}")
            nq = (c1 - c0) // 1024
            for qq in range(nq):
                cs = slice(1024 * qq, 1024 * qq + 1024)
                zq = zh[:, cs]
                nc.scalar.activation(out=zq,
                                     in_=E_h2[:, c0 + 1024 * qq:
                                              c0 + 1024 * qq + 1024],
                                     func=AF.Exp, scale=corr)
                zv = zq.rearrange("p (h w) -> p h w", w=128)
                zs = awork.tile([128, 8], f32, tag="zs", bufs=4)
                nc.vector.tensor_reduce(out=zs, in_=zv, axis=AX.X, op=ALU.add)
                zr = awork.tile([128, 8], f32, tag="zr", bufs=4)
                nc.vector.reciprocal(out=zr, in_=zs)
                nc.vector.tensor_tensor(out=zv, in0=zv,
                                        in1=_bcast_last(zr, 128), op=ALU.mult)
                nc.vector.tensor_tensor(
                    out=ciah[:, cs], in0=zq,
                    in1=xblk_sb[:, c0 + 1024 * qq:c0 + 1024 * qq + 1024],
                    op=ALU.add)
            nc.sync.dma_start(out=cc_h[h][0].ap(), in_=ciah)
            nc.gpsimd.collective_compute(
                "AllGather", ALU.bypass, replica_groups=G4,
                ins=[cc_h[h][0].ap()], outs=[cc_h[h][1].ap()])

        # ================= transposes of y (dup halves) =================
        # brT_dup/x43T_dup: [p, oh, c] with p = w (0-63) and w+64 (dup copy)
        brT_dup = xpool.tile([128, 64, 64], bf16, tag="xpool", name="brT")
        x43T_dup = xpool.tile([128, 64, 64], bf16, tag="xpool", name="x43T")
        id64a = id128[0:64, 0:64]
        for chunk in range(8):
            ps = pst_b()
            for i in range(8):
                oh = 8 * chunk + i
                nc.tensor.transpose(ps[0:64, 64 * i:64 * i + 64],
                                    in_=y_sb[:, oh, :], identity=id64a,
                                    tile_position=(0, 0))
            pv = ps[0:64, :].rearrange("p (a b) -> p a b", a=8)
            csl = slice(8 * chunk, 8 * chunk + 8)
            nc.scalar.activation(out=brT_dup[0:64, csl, :], in_=pv,
                                 func=AF.Relu)
            tmpl = attn.tile([64, 8, 64], bf16, tag="tmpl", bufs=1)
            nc.vector.tensor_scalar_mul(tmpl, pv, 0.2)
            tmpl2 = attn.tile([64, 8, 64], bf16, tag="tmpl2", bufs=1)
            nc.vector.tensor_tensor(out=tmpl2, in0=pv, in1=tmpl, op=ALU.max)
            nc.scalar.activation(out=x43T_dup[0:64, csl, :], in_=tmpl2,
                                 func=AF.Sigmoid)
        nc.sync.dma_start(out=brT_dup[64:128, :, :], in_=brT_dup[0:64, :, :])
        nc.sync.dma_start(out=x43T_dup[64:128, :, :],
                          in_=x43T_dup[0:64, :, :])

        # ================= T1: att2 + a2T (during AllGather) ============
        a2T_all = awork.tile([128, 8, 4, 64], bf16, tag="a2T_all")
        id64b = id128[64:128, 64:128]
        for g in range(8):
            ps2 = pst_f()
            for m in range(4):
                cA, cB = 8 * g + 2 * m, 8 * g + 2 * m + 1
                msl = slice(64 * m, 64 * m + 64)
                nc.tensor.matmul(ps2[0:64, msl], lhsT=brT_dup[0:64, :, cA],
                                 rhs=x43T_dup[0:64, :, cA], start=True,
                                 stop=True, tile_position=(0, 0))
                nc.tensor.matmul(ps2[64:128, msl], lhsT=brT_dup[64:128, :, cB],
                                 rhs=x43T_dup[64:128, :, cB], start=True,
                                 stop=True, tile_position=(64, 64))
            v2 = ps2[:, 0:256].rearrange("p (a b) -> p a b", a=4)
            e2 = attn.tile([128, 4, 64], bf16, tag="e2")
            nc.scalar.activation(out=e2, in_=v2, func=AF.Exp)
            s2 = awork.tile([128, 4], f32, tag="s2", bufs=2)
            nc.vector.tensor_reduce(out=s2, in_=e2, axis=AX.X, op=ALU.add)
            r2 = awork.tile([128, 4], f32, tag="r2", bufs=2)
            nc.vector.reciprocal(out=r2, in_=s2)
            att2sm = attn.tile([128, 4, 64], bf16, tag="att2sm")
            nc.vector.tensor_tensor(out=att2sm, in0=e2, in1=_bcast_last(r2, 64),
                                    op=ALU.mult)
            psTt = pst_b()
            for m in range(4):
                msl = slice(64 * m, 64 * m + 64)
                nc.tensor.transpose(psTt[0:64, msl], in_=att2sm[0:64, m, :],
                                    identity=id64a, tile_position=(0, 0))
                nc.tensor.transpose(psTt[64:128, msl], in_=att2sm[64:128, m, :],
                                    identity=id64b, tile_position=(64, 64))
            nc.vector.tensor_copy(
                out=a2T_all[:, g, :, :],
                in_=psTt[:, 0:256].rearrange("p (a b) -> p a b", a=4))

        # ================= conv(cia) =================
        xpc = []
        for k in range(2):
            xp = convin.tile([128, 130, 128], bf16, tag="convin")
            nc.vector.memset(xp[:, 0:2, :], 0.0)
            for half in range(2):
                src_t = cc_h[half][1]
                for par in range(2):
                    for hh in range(2):
                        src = src_t.ap()[2 * par + hh,
                                         64 * k:64 * k + 64, :] \
                            .rearrange("p (h w) -> p h w", w=128)
                        r0 = 2 + 64 * hh + 32 * half
                        dst = xp[64 * par:64 * par + 64, r0:r0 + 32, :]
                        nc.sync.dma_start(out=dst, in_=src)
            xpc.append(xp)
        x12_sb = convout.tile([64, 64, 64], bf16, tag="convout", name="x12")
        conv_mms(xpc, (0, 1, 5), AF.Relu, x12_sb)
        conv_mms(xpc, (2, 3, 4, 6, 7), AF.Relu, x12_sb)
        psT_cm.__exit__(None, None, None)
        psB_cm.__exit__(None, None, None)
        psTail = ctx.enter_context(tc.tile_pool(name="psTail", bufs=4,
                                                space="PSUM"))
        psU_pool = psTail

        def pst_f():
            return psTail.tile([128, 512], f32, tag="psX", name="psX_f")

        def pst_b():
            return psTail.tile([128, 512], bf16, tag="psX", name="psX_b")

        # ---- x12T_dup transposes ----
        x12T_dup = xpool.tile([128, 64, 64], bf16, tag="xpool", name="x12T")
        for chunk in range(8):
            ps = pst_b()
            for i in range(8):
                oh = 8 * chunk + i
                nc.tensor.transpose(ps[0:64, 64 * i:64 * i + 64],
                                    in_=x12_sb[:, oh, :], identity=id64a,
                                    tile_position=(0, 0))
            nc.vector.tensor_copy(
                out=x12T_dup[0:64, 8 * chunk:8 * chunk + 8, :],
                in_=ps[0:64, :].rearrange("p (a b) -> p a b", a=8))
        nc.sync.dma_start(out=x12T_dup[64:128, :, :], in_=x12T_dup[0:64, :, :])

        # ================= T2: att1, x3_3, S, upsample =================
        for g in range(8):
            def qmm(psd, lt, rh):
                for m in range(4):
                    cA, cB = 8 * g + 2 * m, 8 * g + 2 * m + 1
                    msl = slice(64 * m, 64 * m + 64)
                    nc.tensor.matmul(psd[0:64, msl], lhsT=lt[0:64, :, cA],
                                     rhs=rh(0, m, cA), start=True, stop=True,
                                     tile_position=(0, 0))
                    nc.tensor.matmul(psd[64:128, msl], lhsT=lt[64:128, :, cB],
                                     rhs=rh(64, m, cB), start=True, stop=True,
                                     tile_position=(64, 64))

            # --- att1 (max-shifted softmax) ---
            ps1 = pst_f()
            qmm(ps1, x12T_dup,
                lambda o, m, c: brT_dup[o:o + 64, :, c])
            v1 = ps1[:, 0:256].rearrange("p (a b) -> p a b", a=4)
            nm1 = awork.tile([128, 4], f32, tag="nm1", bufs=2)
            nc.vector.tensor_reduce(out=nm1, in_=v1, axis=AX.X, op=ALU.max,
                                    negate=True)
            lg1 = attn.tile([128, 4, 64], bf16, tag="lg1", bufs=2)
            nc.vector.tensor_tensor(out=lg1, in0=v1, in1=_bcast_last(nm1, 64),
                                    op=ALU.add)
            e1 = attn.tile([128, 4, 64], bf16, tag="e1")
            nc.scalar.activation(out=e1, in_=lg1, func=AF.Exp)
            s1 = awork.tile([128, 4], f32, tag="s1", bufs=2)
            nc.vector.tensor_reduce(out=s1, in_=e1, axis=AX.X, op=ALU.add)
            r1 = awork.tile([128, 4], f32, tag="r1", bufs=2)
            nc.vector.reciprocal(out=r1, in_=s1)
            att1sm = attn.tile([128, 4, 64], bf16, tag="att1sm")
            nc.gpsimd.tensor_tensor(out=att1sm, in0=e1, in1=_bcast_last(r1, 64),
                                    op=ALU.mult)

            # --- x3_3 (shift-free) ---
            ps3 = pst_f()
            qmm(ps3, x12T_dup,
                lambda o, m, c: a2T_all[o:o + 64, g, m, :])
            e3 = attn.tile([128, 4, 64], bf16, tag="e3")
            nc.scalar.activation(out=e3,
                                 in_=ps3[:, 0:256].rearrange("p (a b) -> p a b", a=4),
                                 func=AF.Exp)
            s3 = awork.tile([128, 4], f32, tag="s3", bufs=2)
            nc.vector.tensor_reduce(out=s3, in_=e3, axis=AX.X, op=ALU.add)
            r3 = awork.tile([128, 4], f32, tag="r3", bufs=2)
            nc.vector.reciprocal(out=r3, in_=s3)
            x33sm = attn.tile([128, 4, 64], bf16, tag="x33sm")
            nc.gpsimd.tensor_tensor(out=x33sm, in0=e3, in1=_bcast_last(r3, 64),
                                    op=ALU.mult)

            # --- branch natural layout via transpose ---
            psB2 = pst_b()
            for m in range(4):
                cA, cB = 8 * g + 2 * m, 8 * g + 2 * m + 1
                msl = slice(64 * m, 64 * m + 64)
                nc.tensor.transpose(psB2[0:64, msl], in_=brT_dup[0:64, :, cA],
                                    identity=id64a, tile_position=(0, 0))
                nc.tensor.transpose(psB2[64:128, msl],
                                    in_=brT_dup[64:128, :, cB],
                                    identity=id64b, tile_position=(64, 64))

            # --- S = relu(x33sm + att1sm + br_nat) ---
            t2 = attn.tile([128, 4, 64], bf16, tag="t2", bufs=2)
            nc.gpsimd.tensor_tensor(out=t2, in0=x33sm, in1=att1sm, op=ALU.add)
            t3 = attn.tile([128, 4, 64], bf16, tag="t3", bufs=2)
            nc.vector.tensor_tensor(
                out=t3, in0=t2,
                in1=psB2[:, 0:256].rearrange("p (a b) -> p a b", a=4), op=ALU.add)
            S_sb = attn.tile([128, 4, 64], bf16, tag="S", bufs=2)
            nc.vector.tensor_scalar_max(S_sb, t3, 0.0)

            # --- upsample: W1T = S^T @ R^T directly (lhsT = S as stored) ---
            psWT = psU_pool.tile([128, 4, 128], f32, tag="psU", name="psWT")
            for m in range(4):
                nc.tensor.matmul(psWT[0:64, m, :], lhsT=S_sb[0:64, m, :],
                                 rhs=rt_sb[0:64, :], start=True, stop=True,
                                 tile_position=(0, 0))
                nc.tensor.matmul(psWT[64:128, m, :], lhsT=S_sb[64:128, m, :],
                                 rhs=rt_sb[64:128, :], start=True, stop=True,
                                 tile_position=(64, 64))
            W1T = attn.tile([128, 4, 128], bf16, tag="W1T")
            nc.vector.tensor_copy(out=W1T, in_=psWT)
            # --- U = W1 @ R^T (row-tiled pairs) ---
            psUa = psU_pool.tile([128, 4, 128], f32, tag="psU", name="psUa")
            psUb = psU_pool.tile([128, 4, 128], f32, tag="psU", name="psUb")
            for m in range(4):
                nc.tensor.matmul(psUa[:, m, :], lhsT=W1T[0:64, m, :],
                                 rhs=rt_sb[0:64, :], start=True, stop=True)
                nc.tensor.matmul(psUb[0:64, m, :],
                                 lhsT=W1T[64:128, m, 0:64],
                                 rhs=rt_sb[64:128, :], start=True, stop=True,
                                 tile_position=(64, 0))
                nc.tensor.matmul(psUb[64:128, m, :],
                                 lhsT=W1T[64:128, m, 64:128],
                                 rhs=rt_sb[64:128, :], start=True, stop=True,
                                 tile_position=(64, 64))
            uA = attn.tile([128, 4, 128], bf16, tag="uA")
            nc.vector.tensor_copy(out=uA, in_=psUa)
            uB = attn.tile([128, 4, 128], bf16, tag="uB")
            nc.vector.tensor_copy(out=uB, in_=psUb)
            nc.sync.dma_start(
                out=out_sh.ap()[8 * g:8 * g + 8:2, :, :]
                .rearrange("c h w -> h c w"), in_=uA)
            nc.sync.dma_start(
                out=out_sh.ap()[8 * g + 1:8 * g + 8:2, :, :]
                .rearrange("c h w -> h c w"), in_=uB)

    nc.finalize()
    return nc


def host_prep(inputs):
    """Build per-core in_maps (host does only slicing/transpose/cast)."""
    bfd = ml_dtypes.bfloat16
    x = np.asarray(inputs["x"], np.float32)
    conv_w = np.asarray(inputs["conv_w"], np.float32)
    conv_b = np.asarray(inputs["conv_b"], np.float32)
    g = np.asarray(inputs["bn_gamma"], np.float32)
    be = np.asarray(inputs["bn_beta"], np.float32)
    mu = np.asarray(inputs["bn_mean"], np.float32)
    var = np.asarray(inputs["bn_var"], np.float32)

    scale = (g / np.sqrt(var + EPS)).astype(np.float32)
    shift = ((conv_b - mu) * scale + be).astype(np.float32)
    Xc = x.reshape(C, -1)
    Xr = x.reshape(-1, C)
    RT = np.ascontiguousarray(_resize_mat(H, HO).T).astype(bfd)
    RT2 = np.ascontiguousarray(np.vstack([RT, RT]))
    wtr = conv_w.transpose(1, 0, 2, 3)  # (ci, co, 3, 3)

    in_maps = []
    for p in range(N_CORES):
        b, q = p // 4, p % 4
        xcT_h = np.ascontiguousarray(Xc[:, 4096 * p:4096 * (p + 1)].T).astype(bfd)
        xr_h = np.ascontiguousarray(Xr[4096 * p:4096 * (p + 1), :]).astype(bfd)
        xcm2_h = np.ascontiguousarray(
            Xc[:, 8192 * q:8192 * (q + 1)].reshape(2, 128, 8192)
            .transpose(1, 0, 2)).astype(bfd)
        xblk_h = np.ascontiguousarray(xcm2_h[:, b, :])
        sel_h = np.zeros((128, 2, 128), np.float32)
        sel_h[np.arange(128), b, np.arange(128)] = 1.0
        cperm = np.concatenate([np.arange(0, 128, 2), np.arange(1, 128, 2)])
        wt_h = np.ascontiguousarray(
            wtr[:, 64 * q:64 * q + 64].reshape(2, 128, 64, 9)
            .transpose(1, 0, 3, 2)[cperm]).astype(bfd)
        wneg_h = np.ascontiguousarray(
            -wtr[:, 64 * q:64 * q + 64, :, 0].reshape(2, 128, 64, 3)
            .transpose(1, 0, 3, 2)[cperm]).astype(bfd)
        in_maps.append({
            "xb": x[b].astype(bfd),
            "wneg": wneg_h,
            "xcT": xcT_h,
            "xr": xr_h,
            "xcm2": xcm2_h,
            "xblk": xblk_h,
            "sel": sel_h.astype(bfd),
            "wt": wt_h,
            "scale": scale[64 * q:64 * q + 64].reshape(64, 1),
            "shift": shift[64 * q:64 * q + 64].reshape(64, 1),
            "rt": RT2,
        })
    return in_maps


_NC = None


def _get_nc():
    global _NC
    if _NC is None:
        _NC = build_nc()
    return _NC


def run_device(inputs, trace=False):
    from concourse.bass_utils import run_bass_kernel_spmd
    nc = _get_nc()
    in_maps = host_prep(inputs)
    try:
        res = run_bass_kernel_spmd(nc, in_maps, list(range(N_CORES)),
                                   trace=trace)
    except (ImportError, ModuleNotFoundError):
        # NTFF profiling hook unavailable in this environment
        res = run_bass_kernel_spmd(nc, in_maps, list(range(N_CORES)),
                                   trace=False)
    out = np.zeros((B, C, H, W), np.float32)
    for p in range(N_CORES):
        b, q = p // 4, p % 4
        out[b, 64 * q:64 * q + 64] = res.results[p]["out_sh"].astype(np.float32)
    return out, res


def kernel(**inputs):
    out, _ = run_device(inputs, trace=False)
    return out


# revision 22
# speedup vs baseline: 9472.3576x; 1.0038x over previous
"""Self-contained Bass/Tile SPMD kernel for nn_AIA_1_56049323213170 (8 NeuronCores).

Pipeline (B=2, C=256, H=W=128), all heavy math on-device in bf16/f32-psum:
  M1 = Xc @ Xr (CxC, contraction sharded 8-way + AllReduce)
  a  = rowsoftmax(M1)  (redundant per core, unnormalized + row-recip trick)
  s  = a @ Xc          (sharded: rows by batch, cols by quarter -> (128, 8192))
  rowsoftmax(s) needs only a global row-sum (|s|<=5.5 -> shift-free exp):
       AllReduce of per-core row sums within each batch group of 4 cores
  cia = x + softmax_W(rowsoftmax(s))   (W-softmax local, shift-free)
  AllGather cia within batch group (split in 2 halves for pipelining)
  y   = BN(conv3x3s2(x)+b), x1_2 = relu(BN(conv3x3s2(cia)+b))  (64 out-ch/core)
  branch = relu(y); x4_3 = sigmoid(lrelu(y,.2))
  att1 = rowsoftmax(x1_2 @ branch^T); att2 = rowsoftmax(branch @ x4_3^T)
  x3_3 = rowsoftmax(x1_2 @ att2^T)  (att2/x3_3 shift-free)
  out  = bilinear_up2(relu(x3_3 + att1 + branch)) via R @ S @ R^T

v2: overlap-oriented schedule (conv-x + att2 prep run during the collective
chain), unpadded conv inputs with ragged edge taps (contiguous DMA), and a
PSUM-quadrant-packed attention tail (two channel-pairs per instruction).

Core p: b = p//4 (batch), q = p%4 (quarter; parity=q//2, h-half=q%2).
Each core returns out[b, 64q:64q+64] as bf16; host assembles f32.
"""
import numpy as np
import ml_dtypes

N_CORES = 8
B, C, H, W = 2, 256, 128, 128
HO = WO = 64
EPS = 1e-5


def _resize_mat(n_out, n_in):
    R = np.zeros((n_out, n_in), np.float32)
    scale = n_in / n_out
    for i in range(n_out):
        src = (i + 0.5) * scale - 0.5
        i0 = int(np.floor(src))
        frac = src - i0
        lo = min(max(i0, 0), n_in - 1)
        hi = min(max(i0 + 1, 0), n_in - 1)
        R[i, lo] += 1.0 - frac
        R[i, hi] += frac
    return R


def _ap_of(t):
    import concourse.bass as bass
    if isinstance(t, bass.AP):
        return t
    return t.ap()


def _bcast_last(t, n):
    """AP of tile t broadcast with a 0-step innermost dim of size n."""
    import concourse.bass as bass
    base = _ap_of(t)
    return bass.AP(tensor=base.tensor, offset=base.offset,
                   ap=[list(d) for d in base.ap] + [[0, n]])


def build_nc():
    from contextlib import ExitStack
    import concourse.bass as bass
    import concourse.mybir as mybir
    import concourse.tile as tile
    from concourse import bacc
    from concourse.masks import make_identity

    f32 = mybir.dt.float32
    bf16 = mybir.dt.bfloat16
    AF = mybir.ActivationFunctionType
    AX = mybir.AxisListType
    ALU = mybir.AluOpType

    nc = bacc.Bacc("TRN2", target_bir_lowering=False, debug=False,
                   num_devices=N_CORES)

    # ---- I/O ----
    xb = nc.dram_tensor("xb", [C, H, W], bf16, kind="ExternalInput")
    xcT = nc.dram_tensor("xcT", [4096, 256], bf16, kind="ExternalInput")
    xr = nc.dram_tensor("xr", [4096, 256], bf16, kind="ExternalInput")
    xcm2 = nc.dram_tensor("xcm2", [128, 2, 8192], bf16, kind="ExternalInput")
    xblk = nc.dram_tensor("xblk", [128, 8192], bf16, kind="ExternalInput")
    sel = nc.dram_tensor("sel", [128, 2, 128], bf16, kind="ExternalInput")
    wt = nc.dram_tensor("wt", [128, 2, 9, 64], bf16, kind="ExternalInput")
    scale_d = nc.dram_tensor("scale", [64, 1], f32, kind="ExternalInput")
    shift_d = nc.dram_tensor("shift", [64, 1], f32, kind="ExternalInput")
    rt_d = nc.dram_tensor("rt", [128, 128], bf16, kind="ExternalInput")
    wneg = nc.dram_tensor("wneg", [128, 2, 3, 64], bf16, kind="ExternalInput")
    out_sh = nc.dram_tensor("out_sh", [64, H, W], bf16, kind="ExternalOutput")

    # ---- collective scratch ----
    cc1_in = nc.dram_tensor("cc1_in", [256, 256], f32)
    cc1_out = nc.dram_tensor("cc1_out", [256, 256], f32, addr_space="Shared")
    cc2_in = nc.dram_tensor("cc2_in", [128, 1], f32)
    cc2_out = nc.dram_tensor("cc2_out", [128, 1], f32)
    cca_in = nc.dram_tensor("cca_in", [128, 4096], bf16)
    cca_out = nc.dram_tensor("cca_out", [4, 128, 4096], bf16)
    ccb_in = nc.dram_tensor("ccb_in", [128, 4096], bf16)
    ccb_out = nc.dram_tensor("ccb_out", [4, 128, 4096], bf16)
    G8 = [list(range(8))]
    G4 = [[0, 1, 2, 3], [4, 5, 6, 7]]

    with tile.TileContext(nc) as tc, ExitStack() as ctx:
        consts = ctx.enter_context(tc.tile_pool(name="consts", bufs=1))
        awork = ctx.enter_context(tc.tile_pool(name="awork", bufs=1))
        big = ctx.enter_context(tc.tile_pool(name="big", bufs=3))
        xpool = ctx.enter_context(tc.tile_pool(name="xpool", bufs=3))
        convin = ctx.enter_context(tc.tile_pool(name="convin", bufs=2))
        convout = ctx.enter_context(tc.tile_pool(name="convout", bufs=1))
        attn = ctx.enter_context(tc.tile_pool(name="attn", bufs=2))
        psB_cm = tc.tile_pool(name="psB", bufs=4, space="PSUM")
        psB = psB_cm.__enter__()
        psA_cm = tc.tile_pool(name="psA", bufs=3, space="PSUM")
        psA = psA_cm.__enter__()

        def psa():
            return psA.tile([128, 512], f32, tag="psA", name="psA_t")

        def psbf():
            return psB.tile([128, 512], f32, tag="psB", name="psB_t")

        # ================= constants =================
        id128 = consts.tile([128, 128], bf16, tag="id128")
        make_identity(nc, id128)
        rt_sb = consts.tile([128, 128], bf16, tag="rt")
        nc.sync.dma_start(out=rt_sb, in_=rt_d.ap())
        sel_sb = consts.tile([128, 2, 128], bf16, tag="sel")
        nc.sync.dma_start(out=sel_sb, in_=sel.ap())
        wt_sb = consts.tile([128, 2, 9, 64], bf16, tag="wt")
        nc.sync.dma_start(out=wt_sb, in_=wt.ap())
        scale_sb = consts.tile([64, 1], f32, tag="scale")
        nc.sync.dma_start(out=scale_sb, in_=scale_d.ap())
        shift_sb = consts.tile([64, 1], f32, tag="shift")
        nc.sync.dma_start(out=shift_sb, in_=shift_d.ap())
        wneg_sb = consts.tile([128, 2, 3, 64], bf16, tag="wneg")
        nc.sync.dma_start(out=wneg_sb, in_=wneg.ap())

        # ================= Phase A: M1 (chunked loads) =================
        xcT_sb = big.tile([128, 32, 256], bf16, tag="big8k", name="xcT_sb")
        xr_sb = big.tile([128, 32, 256], bf16, tag="big8k", name="xr_sb")
        xcT_r = xcT.ap().rearrange("(t p) c -> p t c", p=128)
        xr_r = xr.ap().rearrange("(t p) c -> p t c", p=128)
        _chunks = ((0, 4), (4, 8), (8, 16), (16, 24), (24, 32))
        for c0, c1 in _chunks:
            sl = slice(c0, c1)
            nc.sync.dma_start(out=xcT_sb[:, sl, :], in_=xcT_r[:, sl, :])
            nc.sync.dma_start(out=xr_sb[:, sl, :], in_=xr_r[:, sl, :])

        m1ps = [psa(), psa()]
        for c0, c1 in _chunks:
            for t in range(c0, c1):
                for mc in range(2):
                    nc.tensor.matmul(
                        m1ps[mc][:, 0:256],
                        lhsT=xcT_sb[:, t, 128 * mc:128 * mc + 128],
                        rhs=xr_sb[:, t, :],
                        start=(t == 0), stop=(t == 31))
        for mc in range(2):
            m1e = awork.tile([128, 256], f32, tag="m1e", bufs=2)
            nc.vector.tensor_copy(out=m1e, in_=m1ps[mc][:, 0:256])
            nc.sync.dma_start(out=cc1_in.ap()[128 * mc:128 * mc + 128, :],
                              in_=m1e)
        nc.gpsimd.collective_compute(
            "AllReduce", ALU.add, replica_groups=G8,
            ins=[cc1_in.ap()], outs=[cc1_out.ap()])

        # ================= conv helper (rect taps, +2-row top pad) =========
        # xp tiles are [128, 130, 128] (ch-half, 2+ih, iw); rows 0-1 zero.
        # dj==0 taps read col -1 == previous row col 127 (zero row for oh=0,
        # di=0; real data otherwise) -- corrected by negated-weight matmuls.
        def conv_mms(xpads, octiles, act_func, yout):
            import concourse.bass as bass
            for j in octiles:
                ps = psbf()
                first = True
                for k in range(2):
                    xa = xpads[k][:, :, :]
                    pstep = xa.ap[0][0]
                    for t in (4, 0, 1, 2, 3, 5, 6, 7, 8):
                        di, dj = t // 3, t % 3
                        r0 = 16 * j + di + 1
                        rhs = bass.AP(
                            tensor=xa.tensor,
                            offset=xa.offset + 128 * r0 + dj - 1,
                            ap=[[pstep, 128], [256, 8], [2, 64]])
                        nc.tensor.matmul(
                            ps[0:64, :].rearrange("p (a b) -> p a b", a=8),
                            lhsT=wt_sb[:, k, t, :], rhs=rhs,
                            start=first, stop=False)
                        first = False
                    # left-edge corrections (dj==0 taps wrongly read col -1)
                    for di in range(3):
                        r0 = 16 * j + di
                        rhs = bass.AP(
                            tensor=xa.tensor,
                            offset=xa.offset + 128 * r0 + 127,
                            ap=[[pstep, 128], [256, 8]])
                        nc.tensor.matmul(
                            ps[0:64, 0:449:64], lhsT=wneg_sb[:, k, di, :],
                            rhs=rhs, start=False, stop=(k == 1 and di == 2))
                nc.scalar.activation(
                    out=yout[:, 8 * j: 8 * j + 8, :],
                    in_=ps[0:64, :].rearrange("p (a b) -> p a b", a=8),
                    func=act_func, bias=shift_sb, scale=scale_sb)

        # ---- conv(x) part 1 (octiles 0,1) — fills the AR1 wait ----
        xpx = []
        for k in range(2):
            xp = convin.tile([128, 130, 128], bf16, tag="convin")
            nc.vector.memset(xp[:, 0:2, :], 0.0)
            for par in range(2):
                nc.sync.dma_start(
                    out=xp[64 * par:64 * par + 64, 2:66, :],
                    in_=xb.ap()[128 * k + par:128 * k + 128:2, 0:64, :])
            for par in range(2):
                nc.sync.dma_start(
                    out=xp[64 * par:64 * par + 64, 66:130, :],
                    in_=xb.ap()[128 * k + par:128 * k + 128:2, 64:128, :])
            xpx.append(xp)
        # X halves (contraction rows of Xc) + own-batch block
        X01 = [xpool.tile([128, 8192], bf16, tag="xpool", name=f"X{i}")
               for i in range(2)]
        for i in range(2):
            nc.sync.dma_start(out=X01[i], in_=xcm2.ap()[:, i, :])
        xblk_sb = xpool.tile([128, 8192], bf16, tag="xpool", name="xblk")
        nc.sync.dma_start(out=xblk_sb, in_=xblk.ap())

        y_sb = convout.tile([64, 64, 64], bf16, tag="convout", name="y")
        conv_mms(xpx, (0, 1), AF.Identity, y_sb)

        # ================= Phase A2: softmax(a) + aTo ===================
        a_bf = []
        rinv = []
        for ch in range(2):
            a_raw = awork.tile([128, 256], f32, tag="a_raw")
            nc.sync.dma_start(out=a_raw,
                              in_=cc1_out.ap()[128 * ch:128 * ch + 128, :])
            negmax = awork.tile([128, 1], f32, tag="negmax")
            nc.vector.tensor_reduce(out=negmax, in_=a_raw, axis=AX.X,
                                    op=ALU.max, negate=True)
            a_e = awork.tile([128, 256], bf16, tag="a_e", bufs=2)
            asum = awork.tile([128, 1], f32, tag="asum")
            nc.scalar.activation(out=a_e, in_=a_raw, func=AF.Exp,
                                 bias=negmax, scale=1.0, accum_out=asum)
            ri = awork.tile([128, 1], f32, tag="ri", bufs=2)
            nc.vector.reciprocal(out=ri, in_=asum)
            a_bf.append(a_e)
            rinv.append(ri)

        aTo = []
        for c2c in range(2):
            ps = psa()
            for c1c in range(2):
                nc.tensor.matmul(ps[:, 0:128],
                                 lhsT=a_bf[c1c][:, 128 * c2c:128 * c2c + 128],
                                 rhs=sel_sb[:, c1c, :],
                                 start=(c1c == 0), stop=(c1c == 1))
            t_ = awork.tile([128, 128], bf16, tag="aTo", bufs=2)
            nc.vector.tensor_copy(out=t_, in_=ps[:, 0:128])
            aTo.append(t_)
        sel_f = awork.tile([128, 2, 128], f32, tag="sel_f")
        nc.vector.tensor_copy(out=sel_f, in_=sel_sb)
        ps = psa()
        for c1c in range(2):
            nc.tensor.matmul(ps[:, 0:1], lhsT=sel_f[:, c1c, :],
                             rhs=rinv[c1c], start=(c1c == 0), stop=(c1c == 1))
        rinv_own = awork.tile([128, 1], f32, tag="rinv_own")
        nc.vector.tensor_copy(out=rinv_own, in_=ps[:, 0:1])

        # ================= Phase B: M2 -> E = exp(s) =================
        E_h2 = big.tile([128, 8192], bf16, tag="big8k", name="E")
        partials = awork.tile([128, 16], f32, tag="partials")
        for nci in range(16):
            ps = psa()
            for bp in range(2):
                nc.tensor.matmul(ps,
                                 lhsT=aTo[bp],
                                 rhs=X01[bp][:, 512 * nci:512 * nci + 512],
                                 start=(bp == 0), stop=(bp == 1))
            nc.scalar.activation(
                out=E_h2[:, 512 * nci: 512 * nci + 512],
                in_=ps, func=AF.Exp, scale=rinv_own,
                accum_out=partials[:, nci:nci + 1])
        sloc = awork.tile([128, 1], f32, tag="sloc")
        nc.vector.tensor_reduce(out=sloc, in_=partials, axis=AX.X, op=ALU.add)
        nc.sync.dma_start(out=cc2_in.ap(), in_=sloc)
        nc.gpsimd.collective_compute(
            "AllReduce", ALU.add, replica_groups=G4,
            ins=[cc2_in.ap()], outs=[cc2_out.ap()])
        psA_cm.__exit__(None, None, None)
        psT_cm = tc.tile_pool(name="psT", bufs=4, space="PSUM")
        psT = psT_cm.__enter__()

        def pst_f():
            return psT.tile([128, 512], f32, tag="psT", name="psT_f")

        def pst_b():
            return psT.tile([128, 512], bf16, tag="psT", name="psT_b")

        gsum = awork.tile([128, 1], f32, tag="gsum")
        nc.sync.dma_start(out=gsum, in_=cc2_out.ap())
        corr = awork.tile([128, 1], f32, tag="corr")
        nc.vector.reciprocal(out=corr, in_=gsum)

        # ---- conv(x) part 2 (octiles 2..7) ----
        conv_mms(xpx, range(2, 8), AF.Identity, y_sb)

        # ================= z-chain (2 halves) + split AllGather =========
        cc_h = [(cca_in, cca_out), (ccb_in, ccb_out)]
        HSPLIT = 4096
        for h, (c0, c1) in enumerate(((0, HSPLIT), (HSPLIT, 8192))):
            zh = big.tile([128, c1 - c0], bf16, tag="big8k", name=f"z{h}")
            ciah = big.tile([128, c1 - c0], bf16, tag="big8k", name=f"cia{h}")
            nq = (c1 - c0) // 1024
            for qq in range(nq):
                cs = slice(1024 * qq, 1024 * qq + 1024)
                zq = zh[:, cs]
                nc.scalar.activation(out=zq,
                                     in_=E_h2[:, c0 + 1024 * qq:
                                              c0 + 1024 * qq + 1024],
                                     func=AF.Exp, scale=corr)
                zv = zq.rearrange("p (h w) -> p h w", w=128)
                zs = awork.tile([128, 8], f32, tag="zs", bufs=4)
                nc.vector.tensor_reduce(out=zs, in_=zv, axis=AX.X, op=ALU.add)
                zr = awork.tile([128, 8], f32, tag="zr", bufs=4)
                nc.vector.reciprocal(out=zr, in_=zs)
                nc.vector.tensor_tensor(out=zv, in0=zv,
                                        in1=_bcast_last(zr, 128), op=ALU.mult)
                nc.vector.tensor_tensor(
                    out=ciah[:, cs], in0=zq,
                    in1=xblk_sb[:, c0 + 1024 * qq:c0 + 1024 * qq + 1024],
                    op=ALU.add)
            nc.sync.dma_start(out=cc_h[h][0].ap(), in_=ciah)
            nc.gpsimd.collective_compute(
                "AllGather", ALU.bypass, replica_groups=G4,
                ins=[cc_h[h][0].ap()], outs=[cc_h[h][1].ap()])

        # ================= transposes of y (dup halves) =================
        # brT_dup/x43T_dup: [p, oh, c] with p = w (0-63) and w+64 (dup copy)
        brT_dup = xpool.tile([128, 64, 64], bf16, tag="xpool", name="brT")
        x43T_dup = xpool.tile([128, 64, 64], bf16, tag="xpool", name="x43T")
        id64a = id128[0:64, 0:64]
        for chunk in range(8):
            ps = pst_b()
            for i in range(8):
                oh = 8 * chunk + i
                nc.tensor.transpose(ps[0:64, 64 * i:64 * i + 64],
                                    in_=y_sb[:, oh, :], identity=id64a,
                                    tile_position=(0, 0))
            pv = ps[0:64, :].rearrange("p (a b) -> p a b", a=8)
            csl = slice(8 * chunk, 8 * chunk + 8)
            nc.scalar.activation(out=brT_dup[0:64, csl, :], in_=pv,
                                 func=AF.Relu)
            tmpl = attn.tile([64, 8, 64], bf16, tag="tmpl", bufs=1)
            nc.vector.tensor_scalar_mul(tmpl, pv, 0.2)
            tmpl2 = attn.tile([64, 8, 64], bf16, tag="tmpl2", bufs=1)
            nc.vector.tensor_tensor(out=tmpl2, in0=pv, in1=tmpl, op=ALU.max)
            nc.scalar.activation(out=x43T_dup[0:64, csl, :], in_=tmpl2,
                                 func=AF.Sigmoid)
        nc.sync.dma_start(out=brT_dup[64:128, :, :], in_=brT_dup[0:64, :, :])
        nc.sync.dma_start(out=x43T_dup[64:128, :, :],
                          in_=x43T_dup[0:64, :, :])

        # ================= T1: att2 + a2T (during AllGather) ============
        a2T_all = awork.tile([128, 8, 4, 64], bf16, tag="a2T_all")
        id64b = id128[64:128, 64:128]
        for g in range(8):
            ps2 = pst_f()
            for m in range(4):
                cA, cB = 8 * g + 2 * m, 8 * g + 2 * m + 1
                msl = slice(64 * m, 64 * m + 64)
                nc.tensor.matmul(ps2[0:64, msl], lhsT=brT_dup[0:64, :, cA],
                                 rhs=x43T_dup[0:64, :, cA], start=True,
                                 stop=True, tile_position=(0, 0))
                nc.tensor.matmul(ps2[64:128, msl], lhsT=brT_dup[64:128, :, cB],
                                 rhs=x43T_dup[64:128, :, cB], start=True,
                                 stop=True, tile_position=(64, 64))
            v2 = ps2[:, 0:256].rearrange("p (a b) -> p a b", a=4)
            e2 = attn.tile([128, 4, 64], bf16, tag="e2")
            nc.scalar.activation(out=e2, in_=v2, func=AF.Exp)
            s2 = awork.tile([128, 4], f32, tag="s2", bufs=2)
            nc.vector.tensor_reduce(out=s2, in_=e2, axis=AX.X, op=ALU.add)
            r2 = awork.tile([128, 4], f32, tag="r2", bufs=2)
            nc.vector.reciprocal(out=r2, in_=s2)
            att2sm = attn.tile([128, 4, 64], bf16, tag="att2sm")
            nc.vector.tensor_tensor(out=att2sm, in0=e2, in1=_bcast_last(r2, 64),
                                    op=ALU.mult)
            psTt = pst_b()
            for m in range(4):
                msl = slice(64 * m, 64 * m + 64)
                nc.tensor.transpose(psTt[0:64, msl], in_=att2sm[0:64, m, :],
                                    identity=id64a, tile_position=(0, 0))
                nc.tensor.transpose(psTt[64:128, msl], in_=att2sm[64:128, m, :],
                                    identity=id64b, tile_position=(64, 64))
            nc.vector.tensor_copy(
                out=a2T_all[:, g, :, :],
                in_=psTt[:, 0:256].rearrange("p (a b) -> p a b", a=4))

        # ================= conv(cia) =================
        xpc = []
        for k in range(2):
            xp = convin.tile([128, 130, 128], bf16, tag="convin")
            nc.vector.memset(xp[:, 0:2, :], 0.0)
            for half in range(2):
                src_t = cc_h[half][1]
                for par in range(2):
                    for hh in range(2):
                        src = src_t.ap()[2 * par + hh,
                                         64 * k:64 * k + 64, :] \
                            .rearrange("p (h w) -> p h w", w=128)
                        r0 = 2 + 64 * hh + 32 * half
                        dst = xp[64 * par:64 * par + 64, r0:r0 + 32, :]
                        nc.sync.dma_start(out=dst, in_=src)
            xpc.append(xp)
        x12_sb = convout.tile([64, 64, 64], bf16, tag="convout", name="x12")
        conv_mms(xpc, (0, 1, 5), AF.Relu, x12_sb)
        conv_mms(xpc, (2, 3, 4, 6, 7), AF.Relu, x12_sb)
        psT_cm.__exit__(None, None, None)
        psB_cm.__exit__(None, None, None)
        psTail = ctx.enter_context(tc.tile_pool(name="psTail", bufs=4,
                                                space="PSUM"))
        psU_pool = psTail

        def pst_f():
            return psTail.tile([128, 512], f32, tag="psX", name="psX_f",
                               bufs=2)

        def pst_b():
            return psTail.tile([128, 512], bf16, tag="psX", name="psX_b",
                               bufs=2)

        # ---- x12T_dup transposes ----
        x12T_dup = xpool.tile([128, 64, 64], bf16, tag="xpool", name="x12T")
        for chunk in range(8):
            ps = pst_b()
            for i in range(8):
                oh = 8 * chunk + i
                nc.tensor.transpose(ps[0:64, 64 * i:64 * i + 64],
                                    in_=x12_sb[:, oh, :], identity=id64a,
                                    tile_position=(0, 0))
            nc.vector.tensor_copy(
                out=x12T_dup[0:64, 8 * chunk:8 * chunk + 8, :],
                in_=ps[0:64, :].rearrange("p (a b) -> p a b", a=8))
        nc.sync.dma_start(out=x12T_dup[64:128, :, :], in_=x12T_dup[0:64, :, :])

        # ================= T2: att1, x3_3, S, upsample (2 groups/round) ====
        for r in range(4):
            def chans(s):
                g = 2 * r + s // 4
                m = s % 4
                return g, m, 8 * g + 2 * m, 8 * g + 2 * m + 1

            # --- att1 logits (8 quadrant pairs) ---
            ps1 = psTail.tile([128, 512], f32, tag="psX", name="ps1", bufs=2)
            for s in range(8):
                _, _, cA, cB = chans(s)
                msl = slice(64 * s, 64 * s + 64)
                nc.tensor.matmul(ps1[0:64, msl], lhsT=x12T_dup[0:64, :, cA],
                                 rhs=brT_dup[0:64, :, cA], start=True,
                                 stop=True, tile_position=(0, 0))
                nc.tensor.matmul(ps1[64:128, msl], lhsT=x12T_dup[64:128, :, cB],
                                 rhs=brT_dup[64:128, :, cB], start=True,
                                 stop=True, tile_position=(64, 64))
            v1 = ps1.rearrange("p (a b) -> p a b", a=8)
            nm1 = awork.tile([128, 8], f32, tag="nm1", bufs=2)
            nc.vector.tensor_reduce(out=nm1, in_=v1, axis=AX.X, op=ALU.max,
                                    negate=True)
            lg1 = attn.tile([128, 8, 64], bf16, tag="lg1", bufs=1)
            nc.vector.tensor_tensor(out=lg1, in0=v1, in1=_bcast_last(nm1, 64),
                                    op=ALU.add)
            e1 = attn.tile([128, 8, 64], bf16, tag="e1", bufs=1)
            nc.scalar.activation(out=e1, in_=lg1, func=AF.Exp)
            s1 = awork.tile([128, 8], f32, tag="s1", bufs=2)
            nc.vector.tensor_reduce(out=s1, in_=e1, axis=AX.X, op=ALU.add)
            r1 = awork.tile([128, 8], f32, tag="r1", bufs=2)
            nc.vector.reciprocal(out=r1, in_=s1)
            att1sm = attn.tile([128, 8, 64], bf16, tag="att1sm", bufs=1)
            nc.gpsimd.tensor_tensor(out=att1sm, in0=e1, in1=_bcast_last(r1, 64),
                                    op=ALU.mult)

            # --- x3_3 (shift-free) ---
            ps3 = psTail.tile([128, 512], f32, tag="psX", name="ps3", bufs=2)
            for s in range(8):
                g, m, cA, cB = chans(s)
                msl = slice(64 * s, 64 * s + 64)
                nc.tensor.matmul(ps3[0:64, msl], lhsT=x12T_dup[0:64, :, cA],
                                 rhs=a2T_all[0:64, g, m, :], start=True,
                                 stop=True, tile_position=(0, 0))
                nc.tensor.matmul(ps3[64:128, msl], lhsT=x12T_dup[64:128, :, cB],
                                 rhs=a2T_all[64:128, g, m, :], start=True,
                                 stop=True, tile_position=(64, 64))
            e3 = attn.tile([128, 8, 64], bf16, tag="e3", bufs=1)
            nc.scalar.activation(out=e3,
                                 in_=ps3.rearrange("p (a b) -> p a b", a=8),
                                 func=AF.Exp)
            s3 = awork.tile([128, 8], f32, tag="s3", bufs=2)
            nc.vector.tensor_reduce(out=s3, in_=e3, axis=AX.X, op=ALU.add)
            r3 = awork.tile([128, 8], f32, tag="r3", bufs=2)
            nc.vector.reciprocal(out=r3, in_=s3)
            x33sm = attn.tile([128, 8, 64], bf16, tag="x33sm", bufs=1)
            nc.gpsimd.tensor_tensor(out=x33sm, in0=e3, in1=_bcast_last(r3, 64),
                                    op=ALU.mult)

            # --- branch natural layout via transpose ---
            psB2 = psTail.tile([128, 512], bf16, tag="psX", name="psB2",
                               bufs=2)
            for s in range(8):
                _, _, cA, cB = chans(s)
                msl = slice(64 * s, 64 * s + 64)
                nc.tensor.transpose(psB2[0:64, msl], in_=brT_dup[0:64, :, cA],
                                    identity=id64a, tile_position=(0, 0))
                nc.tensor.transpose(psB2[64:128, msl],
                                    in_=brT_dup[64:128, :, cB],
                                    identity=id64b, tile_position=(64, 64))

            # --- S = relu(x33sm + att1sm + br_nat) ---
            t2 = attn.tile([128, 8, 64], bf16, tag="t2", bufs=1)
            nc.gpsimd.tensor_tensor(out=t2, in0=x33sm, in1=att1sm, op=ALU.add)
            t3 = attn.tile([128, 8, 64], bf16, tag="t3", bufs=1)
            nc.vector.tensor_tensor(
                out=t3, in0=t2,
                in1=psB2.rearrange("p (a b) -> p a b", a=8), op=ALU.add)
            S_sb = attn.tile([128, 8, 64], bf16, tag="S", bufs=1)
            nc.vector.tensor_scalar_max(S_sb, t3, 0.0)

            # --- upsample: W1T = S^T @ R^T directly ---
            psWT = psTail.tile([128, 8, 128], f32, tag="psU", name="psWT",
                               bufs=3)
            for s in range(8):
                nc.tensor.matmul(psWT[0:64, s, :], lhsT=S_sb[0:64, s, :],
                                 rhs=rt_sb[0:64, :], start=True, stop=True,
                                 tile_position=(0, 0))
                nc.tensor.matmul(psWT[64:128, s, :], lhsT=S_sb[64:128, s, :],
                                 rhs=rt_sb[64:128, :], start=True, stop=True,
                                 tile_position=(64, 64))
            W1T = attn.tile([128, 8, 128], bf16, tag="W1T", bufs=1)
            nc.scalar.activation(out=W1T, in_=psWT, func=AF.Identity)
            # --- U = W1 @ R^T ---
            psUa = psTail.tile([128, 8, 128], f32, tag="psU", name="psUa",
                               bufs=3)
            psUb = psTail.tile([128, 8, 128], f32, tag="psU", name="psUb",
                               bufs=3)
            for s in range(8):
                nc.tensor.matmul(psUa[:, s, :], lhsT=W1T[0:64, s, :],
                                 rhs=rt_sb[0:64, :], start=True, stop=True)
                nc.tensor.matmul(psUb[0:64, s, :],
                                 lhsT=W1T[64:128, s, 0:64],
                                 rhs=rt_sb[64:128, :], start=True, stop=True,
                                 tile_position=(64, 0))
                nc.tensor.matmul(psUb[64:128, s, :],
                                 lhsT=W1T[64:128, s, 64:128],
                                 rhs=rt_sb[64:128, :], start=True, stop=True,
                                 tile_position=(64, 64))
            uA = attn.tile([128, 8, 128], bf16, tag="uA", bufs=2)
            nc.vector.tensor_copy(out=uA, in_=psUa)
            uB = attn.tile([128, 8, 128], bf16, tag="uB", bufs=1)
            nc.scalar.activation(out=uB, in_=psUb, func=AF.Identity)
            nc.sync.dma_start(
                out=out_sh.ap()[16 * r:16 * r + 16:2, :, :]
                .rearrange("c h w -> h c w"), in_=uA)
            nc.sync.dma_start(
                out=out_sh.ap()[16 * r + 1:16 * r + 16:2, :, :]
                .rearrange("c h w -> h c w"), in_=uB)

    nc.finalize()
    return nc


def host_prep(inputs):
    """Build per-core in_maps (host does only slicing/transpose/cast)."""
    bfd = ml_dtypes.bfloat16
    x = np.asarray(inputs["x"], np.float32)
    conv_w = np.asarray(inputs["conv_w"], np.float32)
    conv_b = np.asarray(inputs["conv_b"], np.float32)
    g = np.asarray(inputs["bn_gamma"], np.float32)
    be = np.asarray(inputs["bn_beta"], np.float32)
    mu = np.asarray(inputs["bn_mean"], np.float32)
    var = np.asarray(inputs["bn_var"], np.float32)

    scale = (g / np.sqrt(var + EPS)).astype(np.float32)
    shift = ((conv_b - mu) * scale + be).astype(np.float32)
    Xc = x.reshape(C, -1)
    Xr = x.reshape(-1, C)
    RT = np.ascontiguousarray(_resize_mat(H, HO).T).astype(bfd)
    RT2 = np.ascontiguousarray(np.vstack([RT, RT]))
    wtr = conv_w.transpose(1, 0, 2, 3)  # (ci, co, 3, 3)

    in_maps = []
    for p in range(N_CORES):
        b, q = p // 4, p % 4
        xcT_h = np.ascontiguousarray(Xc[:, 4096 * p:4096 * (p + 1)].T).astype(bfd)
        xr_h = np.ascontiguousarray(Xr[4096 * p:4096 * (p + 1), :]).astype(bfd)
        xcm2_h = np.ascontiguousarray(
            Xc[:, 8192 * q:8192 * (q + 1)].reshape(2, 128, 8192)
            .transpose(1, 0, 2)).astype(bfd)
        xblk_h = np.ascontiguousarray(xcm2_h[:, b, :])
        sel_h = np.zeros((128, 2, 128), np.float32)
        sel_h[np.arange(128), b, np.arange(128)] = 1.0
        cperm = np.concatenate([np.arange(0, 128, 2), np.arange(1, 128, 2)])
        wt_h = np.ascontiguousarray(
            wtr[:, 64 * q:64 * q + 64].reshape(2, 128, 64, 9)
            .transpose(1, 0, 3, 2)[cperm]).astype(bfd)
        wneg_h = np.ascontiguousarray(
            -wtr[:, 64 * q:64 * q + 64, :, 0].reshape(2, 128, 64, 3)
            .transpose(1, 0, 3, 2)[cperm]).astype(bfd)
        in_maps.append({
            "xb": x[b].astype(bfd),
            "wneg": wneg_h,
            "xcT": xcT_h,
            "xr": xr_h,
            "xcm2": xcm2_h,
            "xblk": xblk_h,
            "sel": sel_h.astype(bfd),
            "wt": wt_h,
            "scale": scale[64 * q:64 * q + 64].reshape(64, 1),
            "shift": shift[64 * q:64 * q + 64].reshape(64, 1),
            "rt": RT2,
        })
    return in_maps


_NC = None


def _get_nc():
    global _NC
    if _NC is None:
        _NC = build_nc()
    return _NC


def run_device(inputs, trace=False):
    from concourse.bass_utils import run_bass_kernel_spmd
    nc = _get_nc()
    in_maps = host_prep(inputs)
    try:
        res = run_bass_kernel_spmd(nc, in_maps, list(range(N_CORES)),
                                   trace=trace)
    except (ImportError, ModuleNotFoundError):
        # NTFF profiling hook unavailable in this environment
        res = run_bass_kernel_spmd(nc, in_maps, list(range(N_CORES)),
                                   trace=False)
    out = np.zeros((B, C, H, W), np.float32)
    for p in range(N_CORES):
        b, q = p // 4, p % 4
        out[b, 64 * q:64 * q + 64] = res.results[p]["out_sh"].astype(np.float32)
    return out, res


def kernel(**inputs):
    out, _ = run_device(inputs, trace=False)
    return out
